# revision 15
# baseline (speedup 1.0000x reference)
"""Trainium2 Bass kernel v2 for the 3-scale anchor DetectionLoss.

Data-parallel over batch: 16 samples -> 8 cores x 2 samples. Host sums
the per-core partial accumulators and applies the global normalizer.

Key structure (per sample):
- Anchor layout [128p, 672]: partition = grid row y; cols = s0 (3 sizes x
  128 x), s1 (3 x 64, rows 0..63), s2 (3 x 32, rows 0..31).
- IOU surrogate q = 64 * inter / (A + B): monotone in IOU, so argmax and
  the pos/neg thresholds (iou>=.5 <=> q>=64/3; iou<.3 <=> q<192/13)
  transfer. inter is separable: inter = fy(y) * fx(x), so the per-box
  pair stage is ONE rank-3 outer-product matmul per scale on the PE
  (lhsT = fy rows, rhs = block-diag fx pre-scaled by 64/(A+B)).
- ACT evacuates PSUM->fp16; DVE keeps a running max (BEST). pos/neg come
  from BEST; invalid (ragged) rows get BEST init 16.0, between the two
  scaled thresholds, so they are neither pos nor neg.
- Hard-negative mining: per-scale binary search for the k-th largest
  masked objectness loss; exact top-k sum via S(>thr) + (k-cnt)*thr.
- cls/loc losses only touch positives: per-partition top-16 positive
  columns are extracted with max8/match_replace/max_index, their data
  gathered via indirect DMA from host-interleaved DRAM tables, the
  matched box found by recomputing the 40-box q-strip per slot, and the
  small [128,16] tiles carry the SmoothL1 + CE math.
"""

import numpy as np
from contextlib import ExitStack

import concourse.bass as bass
import concourse.tile as tile
from concourse import bacc, mybir
from concourse import bass_utils
from concourse import bass_isa

F32 = mybir.dt.float32
F16 = mybir.dt.float16
U8 = mybir.dt.uint8
U16 = mybir.dt.uint16
U32 = mybir.dt.uint32
I32 = mybir.dt.int32
Alu = mybir.AluOpType
Act = mybir.ActivationFunctionType
Ax = mybir.AxisListType

NCORES = 8
SPC = 2
NBOX = 40
P = 128
NSLOT = 16
NITER = 9
QSC = 64.0                  # q scale to keep 1/(A+B) in fp16 normal range
POS_THR = QSC / 3.0         # q >= this  <=> iou >= 0.5
NEG_THR = QSC * 0.3 / 1.3   # q <  this  <=> iou < 0.3
GARB = 16.0                 # between NEG_THR (14.77) and POS_THR (21.33)

# scale: (W, H, fxd col off, fxd width, y-block off, y-width)
SC = [(128, 128, 0, 384, 0, 128), (64, 64, 384, 192, 128, 64),
      (32, 32, 576, 96, 192, 32)]
NANCH = 672                 # anchor cols per sample tile
NF = 20                     # fields per PREDI row
# PREDI fields: 0-3 deltas, 4-6 cls logits, 7 xl, 8 xh, 9 yl, 10 yh,
# 11 acx, 12 rwa, 13 rha, 14 lnwa, 15 lnha, 16 A


def bc_ins(ap, dim, n):
    """Insert a stride-0 dim of size n at position dim."""
    layout = [list(d) for d in ap.ap]
    layout.insert(dim, [0, n])
    return bass.AP(ap.tensor, ap.offset, layout)


def _build_body(tc, aps):
    nc = tc.nc
    dve = nc.vector
    act = nc.scalar
    gp = nc.gpsimd

    with ExitStack() as ctx:
        pc = ctx.enter_context(tc.tile_pool(name="const", bufs=1))
        pp = ctx.enter_context(tc.tile_pool(name="prep", bufs=2))
        pq = ctx.enter_context(tc.tile_pool(name="qpair", bufs=1))
        pt = ctx.enter_context(tc.tile_pool(name="ptrans", bufs=1))
        pqs = ctx.enter_context(tc.tile_pool(name="qpsum", bufs=1,
                                             space="PSUM"))
        pd = ctx.enter_context(tc.tile_pool(name="dense", bufs=2))
        psl = ctx.enter_context(tc.tile_pool(name="slots", bufs=1))
        pmi = ctx.enter_context(tc.tile_pool(name="mine", bufs=2))
        pfin = ctx.enter_context(tc.tile_pool(name="fin", bufs=1))
        pps = ctx.enter_context(tc.tile_pool(name="smallps", bufs=1,
                                             space="PSUM"))

        # ---------- per-kernel constants ----------
        XL3 = pc.tile([120, 224], F32, tag="xl3", name="xl3")
        XH3 = pc.tile([120, 224], F32, tag="xh3", name="xh3")
        YL3 = pc.tile([120, 224], F32, tag="yl3", name="yl3")
        YH3 = pc.tile([120, 224], F32, tag="yh3", name="yh3")
        MSK3 = pc.tile([120, 3], F16, tag="msk3", name="msk3")
        for t, k in ((XL3, "xl3"), (XH3, "xh3"), (YL3, "yl3"),
                     (YH3, "yh3"), (MSK3, "msk3")):
            nc.sync.dma_start(t[:], aps[k])

        ONESC = pc.tile([P, 1], F32, tag="onesc", name="onesc")
        dve.memset(ONESC[:], 1.0)
        ONESR = pc.tile([1, P], F32, tag="onesr", name="onesr")
        dve.memset(ONESR[:], 1.0)

        # key weights 1 - c/2048 (descending, distinct, fp16-exact)
        KEYW = pc.tile([P, NANCH], F16, tag="keyw", name="keyw")
        JIF = pc.tile([P, NSLOT * NBOX], F32, tag="jif", name="jif")
        POF = pc.tile([P, 1], F32, tag="pof", name="pof")
        with tc.tile_pool(name="initscr", bufs=1) as pin:
            kwi = pin.tile([P, NANCH], I32, tag="kwi", name="kwi")
            gp.iota(kwi[:], [[1, NANCH]], base=0, channel_multiplier=0)
            kwf = pin.tile([P, NANCH], F32, tag="kwf", name="kwf")
            dve.tensor_copy(kwf[:], kwi[:])
            dve.tensor_scalar(KEYW[:], kwf[:], -1.0 / 2048.0, 1.0,
                              Alu.mult, Alu.add)
            ji = pin.tile([P, NSLOT * NBOX], I32, tag="ji", name="ji")
            gp.iota(ji[:], [[0, NSLOT], [1, NBOX]], base=0,
                    channel_multiplier=0)
            dve.tensor_copy(JIF[:], ji[:])
            pofi = pin.tile([P, 1], I32, tag="pofi", name="pofi")
            gp.iota(pofi[:], [[1, 1]], base=0, channel_multiplier=NANCH)
            dve.tensor_copy(POF[:], pofi[:])      # p * 672

        PART = pfin.tile([P, 18], F32, tag="part", name="part")
        dve.memset(PART[:], 0.0)

        for b in range(SPC):
            pb = b * 8   # PART col base: 0:objpos 1:cls 2:loc 3-5:npos3
            #              6(row0 only):k3x3? -> use cols 6,7 specially

            # ---------- box-dependent prep ----------
            BOXC = pp.tile([120, 4], F32, tag="boxc", name=f"boxc{b}")
            nc.sync.dma_start(BOXC[:], aps["boxc"][b])
            SC3 = pp.tile([120, 3], F32, tag="sc3", name=f"sc3{b}")
            nc.sync.dma_start(SC3[:], aps["sc3"][b])

            # fy per (j,a): [120, 224] fp16
            t1 = pp.tile([120, 224], F32, tag="t1", name=f"t1{b}")
            t2 = pp.tile([120, 224], F32, tag="t2", name=f"t2{b}")
            FYJ = pp.tile([120, 224], F16, tag="fyj", name=f"fyj{b}")
            dve.tensor_scalar(t1[:], YH3[:], BOXC[:, 3:4], None, Alu.min)
            dve.tensor_scalar(t2[:], YL3[:], BOXC[:, 2:3], None, Alu.max)
            dve.tensor_tensor(t1[:], t1[:], t2[:], Alu.subtract)
            act.activation(FYJ[:], t1[:], Act.Relu)
            # fx, scaled by 64/(A+B) per scale block
            fx1 = pp.tile([120, 224], F32, tag="fx1", name=f"fx1{b}")
            fx2 = pp.tile([120, 224], F32, tag="fx2", name=f"fx2{b}")
            FXS = pp.tile([120, 224], F16, tag="fxs", name=f"fxs{b}")
            dve.tensor_scalar(fx1[:], XH3[:], BOXC[:, 1:2], None, Alu.min)
            dve.tensor_scalar(fx2[:], XL3[:], BOXC[:, 0:1], None, Alu.max)
            dve.tensor_tensor(fx1[:], fx1[:], fx2[:], Alu.subtract)
            xo = 0
            for s, (W, H, co, cw, yo, yw) in enumerate(SC):
                act.activation(FXS[:, xo:xo + W], fx1[:, xo:xo + W],
                               Act.Relu, scale=SC3[:, s:s + 1])
                xo += W
            # block-diag expand: FXD[p, (a', x)] = FXS[p, x] * MSK3[p, a']
            FXD = pp.tile([120, NANCH], F16, tag="fxd", name=f"fxd{b}")
            xo = 0
            for s, (W, H, co, cw, yo, yw) in enumerate(SC):
                src = bc_ins(FXS[:, xo:xo + W], 1, 3)
                msk = MSK3[:].to_broadcast([120, 3, W])
                dve.tensor_tensor(
                    FXD[:, co:co + cw].rearrange("p (a x) -> p a x", a=3),
                    src, msk, Alu.mult)
                xo += W

            # ---------- dense obj logits ----------
            POBJ = pd.tile([P, NANCH], F32, tag="pobj", name=f"pobj{b}")
            dve.memset(POBJ[64:128, 384:576], 0.0)
            dve.memset(POBJ[32:64, 576:672], 0.0)
            dve.memset(POBJ[64:128, 576:672], 0.0)
            preds = [aps["pred0"], aps["pred1"], aps["pred2"]]
            for s, (W, H, co, cw, yo, yw) in enumerate(SC):
                for a in range(3):
                    nc.sync.dma_start(
                        POBJ[0:H, co + a * W: co + (a + 1) * W],
                        preds[s][b, a * 8 + 4])

            # ---------- pair stage ----------
            BESTe = pd.tile([P, NANCH], F16, tag="beste", name=f"beste{b}")
            BESTo = pd.tile([P, NANCH], F16, tag="besto", name=f"besto{b}")
            for t in (BESTe, BESTo):
                dve.memset(t[:, 0:384], 0.0)
                dve.memset(t[0:64, 384:576], 0.0)
                dve.memset(t[0:32, 576:672], 0.0)
                dve.memset(t[64:128, 384:576], GARB)
                dve.memset(t[32:64, 576:672], GARB)
                dve.memset(t[64:128, 576:672], GARB)

            # PE operands must sit at base partition 0: reshuffle the
            # [120, *] (j,a)-row tiles into [3, boxes*cols] chunks via DMA.
            CB = 10                      # boxes per chunk
            for ch in range(NBOX // CB):
                FYT = pt.tile([3, CB * 224], F16, tag=f"fyt{ch % 2}",
                              name=f"fyt{b}_{ch}")
                FXT = pt.tile([3, CB * NANCH], F16, tag=f"fxt{ch % 2}",
                              name=f"fxt{b}_{ch}")
                for a in range(3):
                    rows = slice(40 * a + CB * ch, 40 * a + CB * (ch + 1))
                    nc.scalar.dma_start(FYT[a:a + 1, :], FYJ[rows, :])
                    nc.scalar.dma_start(FXT[a:a + 1, :], FXD[rows, :])
                for jj in range(CB):
                    j = ch * CB + jj
                    yo = jj * 224
                    xo = jj * NANCH
                    psA = pqs.tile([P, 384], F32, tag=f"psA{j % 3}",
                                   name=f"psA{b}_{j}")
                    psB = pqs.tile([64, 288], F32, tag=f"psB{j % 3}",
                                   name=f"psB{b}_{j}")
                    nc.tensor.matmul(psA[:], FYT[0:3, yo:yo + 128],
                                     FXT[0:3, xo:xo + 384],
                                     start=True, stop=True)
                    nc.tensor.matmul(psB[0:64, 0:192],
                                     FYT[0:3, yo + 128:yo + 192],
                                     FXT[0:3, xo + 384:xo + 576],
                                     start=True, stop=True)
                    nc.tensor.matmul(psB[0:32, 192:288],
                                     FYT[0:3, yo + 192:yo + 224],
                                     FXT[0:3, xo + 576:xo + 672],
                                     start=True, stop=True)
                    QA = pq.tile([P, 384], F16, tag=f"qa{j % 3}",
                                 name=f"qa{b}_{j}")
                    QB = pq.tile([64, 288], F16, tag=f"qb{j % 3}",
                                 name=f"qb{b}_{j}")
                    act.activation(QA[:], psA[:], Act.Copy)
                    act.activation(QB[0:64, 0:192], psB[0:64, 0:192],
                                   Act.Copy)
                    act.activation(QB[0:32, 192:288], psB[0:32, 192:288],
                                   Act.Copy)
                    acc = BESTe if j % 2 == 0 else BESTo
                    dve.tensor_tensor(acc[:, 0:384], acc[:, 0:384], QA[:],
                                      Alu.max)
                    dve.tensor_tensor(acc[0:64, 384:576],
                                      acc[0:64, 384:576],
                                      QB[0:64, 0:192], Alu.max)
                    dve.tensor_tensor(acc[0:32, 576:672],
                                      acc[0:32, 576:672],
                                      QB[0:32, 192:288], Alu.max)
            BEST = pd.tile([P, NANCH], F16, tag="best", name=f"best{b}")
            dve.tensor_tensor(BEST[:], BESTe[:], BESTo[:], Alu.max)

            # ---------- masks / dense losses ----------
            POS = pd.tile([P, NANCH], F16, tag="pos", name=f"pos{b}")
            NEG = pd.tile([P, NANCH], F16, tag="neg", name=f"neg{b}")
            dve.tensor_scalar(POS[:], BEST[:], POS_THR, None, Alu.is_ge)
            dve.tensor_scalar(NEG[:], BEST[:], NEG_THR, None, Alu.is_lt)

            # softplus(x) = relu(x) + ln(1 + exp(-|x|)), in-place chains
            AX = pd.tile([P, NANCH], F32, tag="ax", name=f"ax{b}")
            SP = pd.tile([P, NANCH], F32, tag="sp", name=f"sp{b}")
            act.activation(AX[:], POBJ[:], Act.Abs)
            act.activation(AX[:], AX[:], Act.Exp, scale=-1.0)
            act.activation(AX[:], AX[:], Act.Ln, bias=1.0)
            act.activation(SP[:], POBJ[:], Act.Relu)
            dve.tensor_tensor(SP[:], SP[:], AX[:], Alu.add)
            # obj positive part: sum pos * (SP - x); POBJ becomes (SP - x)
            dve.tensor_tensor(POBJ[:], SP[:], POBJ[:], Alu.subtract)
            scr = pd.tile([P, NANCH], F32, tag="scr", name=f"scr{b}")
            dve.tensor_tensor(scr[:], POBJ[:], POS[:], Alu.mult)
            dve.tensor_scalar(AX[:], scr[:], 0.0, 0.0, Alu.add, Alu.add,
                              accum_out=PART[:, pb:pb + 1])
            # NEGL = NEG * SP (fp16)
            NEGL = pd.tile([P, NANCH], F16, tag="negl", name=f"negl{b}")
            dve.tensor_tensor(NEGL[:], NEG[:], SP[:], Alu.mult)

            # per-scale pos/neg counts
            mc16a = pd.tile([P, NANCH], F16, tag="mc16a", name=f"mc16a{b}")
            CNT = pmi.tile([P, 8], F32, tag="cnt", name=f"cnt{b}")
            for s, (W, H, co, cw, yo, yw) in enumerate(SC):
                blk = slice(co, co + cw)
                dve.tensor_scalar(mc16a[:, blk], POS[:, blk], 0.0, 0.0,
                                  Alu.add, Alu.add,
                                  accum_out=CNT[:, s:s + 1])
                dve.tensor_scalar(mc16a[:, blk], NEG[:, blk], 0.0, 0.0,
                                  Alu.add, Alu.add,
                                  accum_out=CNT[:, 4 + s:5 + s])
            NPOS3 = pmi.tile([P, 3], F32, tag="npos3", name=f"npos3{b}")
            NNEG3 = pmi.tile([P, 3], F32, tag="nneg3", name=f"nneg3{b}")
            gp.partition_all_reduce(NPOS3[:], CNT[:, 0:3], P,
                                    bass_isa.ReduceOp.add)
            gp.partition_all_reduce(NNEG3[:], CNT[:, 4:7], P,
                                    bass_isa.ReduceOp.add)
            dve.tensor_copy(PART[0:1, pb + 3:pb + 6], NPOS3[0:1, :])

            # ---------- mining: per-scale k-th threshold ----------
            K3 = pmi.tile([P, 3], F32, tag="k3", name=f"k3{b}")
            dve.tensor_scalar(K3[:], NPOS3[:], 1.0, 3.0, Alu.max, Alu.mult)
            dve.tensor_tensor(K3[:], K3[:], NNEG3[:], Alu.min)
            HI3 = pmi.tile([P, 3], F32, tag="hi3", name=f"hi3{b}")
            LO3 = pmi.tile([P, 3], F32, tag="lo3", name=f"lo3{b}")
            MID3 = pmi.tile([P, 3], F32, tag="mid3", name=f"mid3{b}")
            CP3 = pmi.tile([P, 3], F32, tag="cp3", name=f"cp3{b}")
            CT3 = pmi.tile([P, 3], F32, tag="ct3", name=f"ct3{b}")
            GTK = pmi.tile([P, 3], U8, tag="gtk", name=f"gtk{b}")
            LEK = pmi.tile([P, 3], U8, tag="lek", name=f"lek{b}")
            RM3 = pmi.tile([P, 3], F32, tag="rm3", name=f"rm3{b}")
            for s, (W, H, co, cw, yo, yw) in enumerate(SC):
                dve.tensor_reduce(RM3[:, s:s + 1], NEGL[:, co:co + cw],
                                  Ax.X, Alu.max)
            gp.partition_all_reduce(HI3[:], RM3[:], P,
                                    bass_isa.ReduceOp.max)
            dve.memset(LO3[:], 0.0)
            mc16 = pd.tile([P, NANCH], F16, tag="mc16", name=f"mc16{b}")
            for it in range(NITER):
                dve.tensor_tensor(MID3[:], LO3[:], HI3[:], Alu.add)
                dve.tensor_scalar(MID3[:], MID3[:], 0.5, None, Alu.mult)
                for s, (W, H, co, cw, yo, yw) in enumerate(SC):
                    blk = slice(co, co + cw)
                    dve.tensor_scalar(mc16[:, blk], NEGL[:, blk],
                                      MID3[:, s:s + 1], 0.0, Alu.is_gt,
                                      Alu.add, accum_out=CP3[:, s:s + 1])
                gp.partition_all_reduce(CT3[:], CP3[:], P,
                                        bass_isa.ReduceOp.add)
                dve.tensor_tensor(GTK[:], CT3[:], K3[:], Alu.is_gt)
                dve.tensor_tensor(LEK[:], CT3[:], K3[:], Alu.is_le)
                dve.copy_predicated(LO3[:], GTK[:], MID3[:])
                dve.copy_predicated(HI3[:], LEK[:], MID3[:])
            # top-k sum = S(>thr) + (k - cnt(>thr)) * thr ; thr = HI3
            SG3 = pmi.tile([P, 3], F32, tag="sg3", name=f"sg3{b}")
            for s, (W, H, co, cw, yo, yw) in enumerate(SC):
                blk = slice(co, co + cw)
                dve.tensor_scalar(mc16[:, blk], NEGL[:, blk],
                                  HI3[:, s:s + 1], 0.0, Alu.is_gt,
                                  Alu.add, accum_out=CP3[:, s:s + 1])
                dve.tensor_tensor(mc16[:, blk], NEGL[:, blk], mc16[:, blk],
                                  Alu.mult)
                dve.tensor_scalar(mc16[:, blk], mc16[:, blk], 0.0, 0.0,
                                  Alu.add, Alu.add,
                                  accum_out=SG3[:, s:s + 1])
            gp.partition_all_reduce(CT3[:], CP3[:], P,
                                    bass_isa.ReduceOp.add)
            # per-partition SG3 partials summed via PART (full column)
            dve.tensor_copy(PART[:, pb + 6:pb + 7],
                            SG3[:, 0:1])
            dve.tensor_tensor(PART[:, pb + 6:pb + 7], PART[:, pb + 6:pb + 7],
                              SG3[:, 1:2], Alu.add)
            dve.tensor_tensor(PART[:, pb + 6:pb + 7], PART[:, pb + 6:pb + 7],
                              SG3[:, 2:3], Alu.add)
            # (k - cnt) * thr + k  -> row0 only (bcast-identical values)
            TK = pmi.tile([P, 3], F32, tag="tk", name=f"tk{b}")
            dve.tensor_tensor(TK[:], K3[:], CT3[:], Alu.subtract)
            dve.tensor_tensor(TK[:], TK[:], HI3[:], Alu.mult)
            dve.tensor_copy(PART[0:1, pb + 7:pb + 8], TK[0:1, 0:1])
            dve.tensor_tensor(PART[0:1, pb + 7:pb + 8],
                              PART[0:1, pb + 7:pb + 8], TK[0:1, 1:2], Alu.add)
            dve.tensor_tensor(PART[0:1, pb + 7:pb + 8],
                              PART[0:1, pb + 7:pb + 8], TK[0:1, 2:3], Alu.add)
            KS = pmi.tile([P, 1], F32, tag="ks", name=f"ks{b}")
            dve.tensor_copy(KS[:], K3[:, 0:1])
            dve.tensor_tensor(KS[:], KS[:], K3[:, 1:2], Alu.add)
            dve.tensor_tensor(KS[:], KS[:], K3[:, 2:3], Alu.add)

            # ---------- positive slots ----------
            KEY = psl.tile([P, NANCH], F16, tag="key", name=f"key{b}")
            dve.tensor_tensor(KEY[:], POS[:], KEYW[:], Alu.mult)
            K8a = psl.tile([P, 8], F16, tag="k8a", name=f"k8a{b}")
            K8b = psl.tile([P, 8], F16, tag="k8b", name=f"k8b{b}")
            IX8a = psl.tile([P, 8], U16, tag="ix8a", name=f"ix8a{b}")
            IX8b = psl.tile([P, 8], U16, tag="ix8b", name=f"ix8b{b}")
            KEY2 = psl.tile([P, NANCH], F16, tag="key2", name=f"key2{b}")
            dve.max(K8a[:], KEY[:])
            dve.max_index(IX8a[:], K8a[:], KEY[:])
            dve.match_replace(KEY2[:], K8a[:], KEY[:], -1.0)
            dve.max(K8b[:], KEY2[:])
            dve.max_index(IX8b[:], K8b[:], KEY2[:])
            VAL = psl.tile([P, NSLOT], F16, tag="val", name=f"val{b}")
            dve.tensor_scalar(VAL[:, 0:8], K8a[:], 0.0, None, Alu.is_gt)
            dve.tensor_scalar(VAL[:, 8:16], K8b[:], 0.0, None, Alu.is_gt)
            COLU = psl.tile([P, NSLOT], U32, tag="colu", name=f"colu{b}")
            dve.tensor_copy(COLU[:, 0:8], IX8a[:])
            dve.tensor_copy(COLU[:, 8:16], IX8b[:])
            COLF = psl.tile([P, NSLOT], F32, tag="colf", name=f"colf{b}")
            dve.tensor_copy(COLF[:], COLU[:])

            # gather PREDI rows (pred + geometry) at p*672+col, per slot
            OFFP = psl.tile([P, NSLOT], F32, tag="offp", name=f"offp{b}")
            dve.tensor_scalar(OFFP[:], COLF[:], POF[:, 0:1], 0.0, Alu.add,
                              Alu.add)
            OFFPU = psl.tile([P, NSLOT], U32, tag="offpu", name=f"offpu{b}")
            dve.tensor_copy(OFFPU[:], OFFP[:])
            GSA = psl.tile([P, NSLOT * NF], F32, tag="gsa", name=f"gsa{b}")
            for s in range(NSLOT):
                ofs = psl.tile([P, 1], U32, tag=f"ofs{s}", name=f"ofs{b}_{s}")
                dve.tensor_copy(ofs[:], OFFPU[:, s:s + 1])
                gp.indirect_dma_start(
                    out=GSA[:, s * NF:(s + 1) * NF], out_offset=None,
                    in_=aps[f"predi{b}"][:],
                    in_offset=bass.IndirectOffsetOnAxis(ap=ofs[:], axis=0))
            # transpose (slot, field) -> (field, slot) in one strided copy
            GT = psl.tile([P, NF * NSLOT], F32, tag="gt", name=f"gt{b}")
            dve.tensor_copy(GT[:].rearrange("p (f s) -> p s f", s=NSLOT),
                            GSA[:].rearrange("p (s f) -> p s f", f=NF))

            def fld(fi, name):
                return GT[:, fi * NSLOT:(fi + 1) * NSLOT]

            # strip inputs
            XLs = fld(7, "xls")
            XHs = fld(8, "xhs")
            YLs = fld(9, "yls")
            YHs = fld(10, "yhs")
            AAs = fld(16, "aas")

            # box coord broadcast [128, 200] via PE (bx1 bx2 by1 by2 barea)
            bbp = pps.tile([P, 200], F32, tag="bbp", name=f"bbp{b}")
            BROW = psl.tile([1, 200], F32, tag="brow", name=f"brow{b}")
            nc.sync.dma_start(BROW[:], aps["bbrow"][b])
            nc.tensor.matmul(bbp[:], ONESR[:], BROW[:], start=True,
                             stop=True)
            BB = psl.tile([P, 200], F32, tag="bb", name=f"bb{b}")
            act.activation(BB[:], bbp[:], Act.Copy)

            # q strip [128, 16*40] fp32
            SJ = NSLOT * NBOX

            def strip_ov(name, lo_ap, hi_ap, blo, bhi):
                m1 = psl.tile([P, SJ], F32, tag=f"{name}1", name=f"{name}1{b}")
                m2 = psl.tile([P, SJ], F32, tag=f"{name}2", name=f"{name}2{b}")
                v3 = m1[:].rearrange("p (s j) -> p s j", j=NBOX)
                v4 = m2[:].rearrange("p (s j) -> p s j", j=NBOX)
                dve.tensor_tensor(v3, hi_ap, bhi, Alu.min)
                dve.tensor_tensor(v4, lo_ap, blo, Alu.max)
                dve.tensor_tensor(m1[:], m1[:], m2[:], Alu.subtract)
                r = psl.tile([P, SJ], F32, tag=f"{name}r", name=f"{name}r{b}")
                act.activation(r[:], m1[:], Act.Relu)
                return r

            xl_b = XLs.to_broadcast([P, NSLOT, NBOX])
            xh_b = XHs.to_broadcast([P, NSLOT, NBOX])
            yl_b = YLs.to_broadcast([P, NSLOT, NBOX])
            yh_b = YHs.to_broadcast([P, NSLOT, NBOX])
            bx1_b = bc_ins(BB[:, 0:40], 1, NSLOT)
            bx2_b = bc_ins(BB[:, 40:80], 1, NSLOT)
            by1_b = bc_ins(BB[:, 80:120], 1, NSLOT)
            by2_b = bc_ins(BB[:, 120:160], 1, NSLOT)
            FXP = strip_ov("fx", xl_b, xh_b, bx1_b, bx2_b)
            FYP = strip_ov("fy", yl_b, yh_b, by1_b, by2_b)
            # srec = QSC / (A + barea)
            ABJ = psl.tile([P, SJ], F32, tag="abj", name=f"abj{b}")
            dve.tensor_tensor(ABJ[:].rearrange("p (s j) -> p s j", j=NBOX),
                              AAs.to_broadcast([P, NSLOT, NBOX]),
                              bc_ins(BB[:, 160:200], 1, NSLOT), Alu.add)
            SRJ = psl.tile([P, SJ], F32, tag="srj", name=f"srj{b}")
            dve.reciprocal_approx_fast(SRJ[:], ABJ[:])
            QST = psl.tile([P, SJ], F32, tag="qst", name=f"qst{b}")
            dve.tensor_tensor(QST[:], FXP[:], FYP[:], Alu.mult)
            dve.tensor_tensor(QST[:], QST[:], SRJ[:], Alu.mult)
            # argmax-first over j
            BQ = psl.tile([P, NSLOT], F32, tag="bq", name=f"bq{b}")
            dve.tensor_reduce(BQ[:], QST[:].rearrange(
                "p (s j) -> p s j", j=NBOX), Ax.X, Alu.max)
            MSKJ = psl.tile([P, SJ], U8, tag="mskj", name=f"mskj{b}")
            dve.tensor_tensor(MSKJ[:].rearrange("p (s j) -> p s j", j=NBOX),
                              QST[:].rearrange("p (s j) -> p s j", j=NBOX),
                              BQ[:].to_broadcast([P, NSLOT, NBOX]),
                              Alu.is_ge)
            JM = psl.tile([P, SJ], F32, tag="jm", name=f"jm{b}")
            dve.memset(JM[:], 99.0)
            dve.copy_predicated(JM[:], MSKJ[:], JIF[:])
            JF = psl.tile([P, NSLOT], F32, tag="jf", name=f"jf{b}")
            dve.tensor_reduce(JF[:], JM[:].rearrange(
                "p (s j) -> p s j", j=NBOX), Ax.X, Alu.min)
            JU = psl.tile([P, NSLOT], U32, tag="ju", name=f"ju{b}")
            dve.tensor_copy(JU[:], JF[:])
            # gather matched box rows per slot
            BVA = psl.tile([P, NSLOT * 8], F32, tag="bva", name=f"bva{b}")
            for s in range(NSLOT):
                ofj = psl.tile([P, 1], U32, tag=f"ofj{s}", name=f"ofj{b}_{s}")
                dve.tensor_copy(ofj[:], JU[:, s:s + 1])
                gp.indirect_dma_start(
                    out=BVA[:, s * 8:(s + 1) * 8], out_offset=None,
                    in_=aps[f"boxt{b}"][:],
                    in_offset=bass.IndirectOffsetOnAxis(ap=ofj[:], axis=0))
            BVT = psl.tile([P, 8 * NSLOT], F32, tag="bvt", name=f"bvt{b}")
            dve.tensor_copy(BVT[:].rearrange("p (f s) -> p s f", s=NSLOT),
                            BVA[:].rearrange("p (s f) -> p s f", f=8))

            def bfld(fi, name):
                return BVT[:, fi * NSLOT:(fi + 1) * NSLOT]

            BCXs = bfld(0, "bcxs")
            BCYs = bfld(1, "bcys")
            LNWs = bfld(2, "lnws")
            LNHs = bfld(3, "lnhs")
            LABs = bfld(4, "labs")

            def st(name):
                return psl.tile([P, NSLOT], F32, tag=name, name=f"{name}{b}")

            def pfld(fi, name):
                return GT[:, fi * NSLOT:(fi + 1) * NSLOT]

            # ---------- loc loss on slots ----------
            ACXs = fld(11, "acxs")
            RWAs = fld(12, "rwas")
            RHAs = fld(13, "rhas")
            LNWAs = fld(14, "lnwas")
            LNHAs = fld(15, "lnhas")
            ACYs = st("acys")
            dve.tensor_tensor(ACYs[:], YLs, YHs, Alu.add)
            dve.tensor_scalar(ACYs[:], ACYs[:], 0.5, None, Alu.mult)
            encs = []
            e0 = st("e0")
            dve.tensor_tensor(e0[:], BCXs, ACXs, Alu.subtract)
            dve.tensor_tensor(e0[:], e0[:], RWAs, Alu.mult)
            encs.append(e0)
            e1 = st("e1")
            dve.tensor_tensor(e1[:], BCYs, ACYs[:], Alu.subtract)
            dve.tensor_tensor(e1[:], e1[:], RHAs, Alu.mult)
            encs.append(e1)
            e2 = st("e2")
            dve.tensor_tensor(e2[:], LNWs, LNWAs, Alu.subtract)
            encs.append(e2)
            e3 = st("e3")
            dve.tensor_tensor(e3[:], LNHs, LNHAs, Alu.subtract)
            encs.append(e3)
            SL = st("sl")
            first = True
            for c in range(4):
                pd_c = pfld(c, f"pd{c}")
                d = st(f"d{c}")
                dve.tensor_tensor(d[:], pd_c, encs[c][:], Alu.subtract)
                ad = st(f"ad{c}")
                act.activation(ad[:], d[:], Act.Abs)
                mm = st(f"mm{c}")
                dve.tensor_scalar(mm[:], ad[:], 1.0, None, Alu.min)
                q1 = st(f"q1{c}")
                dve.tensor_tensor(q1[:], mm[:], mm[:], Alu.mult)
                dve.tensor_scalar(q1[:], q1[:], 0.5, None, Alu.mult)
                u1 = st(f"u1{c}")
                dve.tensor_tensor(u1[:], ad[:], mm[:], Alu.subtract)
                dve.tensor_tensor(q1[:], q1[:], u1[:], Alu.add)
                if first:
                    dve.tensor_copy(SL[:], q1[:])
                    first = False
                else:
                    dve.tensor_tensor(SL[:], SL[:], q1[:], Alu.add)
            lscr = st("lscr")
            dve.scalar_tensor_tensor(lscr[:], SL[:], 0.0, VAL[:], Alu.add,
                                     Alu.mult,
                                     accum_out=PART[:, pb + 2:pb + 3])

            # ---------- cls loss on slots ----------
            c0 = pfld(4, "c0f")
            c1 = pfld(5, "c1f")
            c2 = pfld(6, "c2f")
            mx = st("mx")
            dve.tensor_tensor(mx[:], c0, c1, Alu.max)
            dve.tensor_tensor(mx[:], mx[:], c2, Alu.max)
            ssum = st("ssum")
            first = True
            for ci, cap in enumerate((c0, c1, c2)):
                dd = st(f"dd{ci}")
                dve.tensor_tensor(dd[:], cap, mx[:], Alu.subtract)
                ee = st(f"ee{ci}")
                act.activation(ee[:], dd[:], Act.Exp)
                if first:
                    dve.tensor_copy(ssum[:], ee[:])
                    first = False
                else:
                    dve.tensor_tensor(ssum[:], ssum[:], ee[:], Alu.add)
            lse = st("lse")
            act.activation(lse[:], ssum[:], Act.Ln)
            dve.tensor_tensor(lse[:], lse[:], mx[:], Alu.add)
            pick = st("pick")
            dve.tensor_copy(pick[:], c0)
            m1u = psl.tile([P, NSLOT], U8, tag="m1u", name=f"m1u{b}")
            m2u = psl.tile([P, NSLOT], U8, tag="m2u", name=f"m2u{b}")
            dve.tensor_scalar(m1u[:], LABs, 2.0, None, Alu.is_equal)
            dve.tensor_scalar(m2u[:], LABs, 3.0, None, Alu.is_equal)
            dve.copy_predicated(pick[:], m1u[:], c1)
            dve.copy_predicated(pick[:], m2u[:], c2)
            ce = st("ce")
            dve.tensor_tensor(ce[:], lse[:], pick[:], Alu.subtract)
            cscr = st("cscr")
            dve.scalar_tensor_tensor(cscr[:], ce[:], 0.0, VAL[:], Alu.add,
                                     Alu.mult,
                                     accum_out=PART[:, pb + 1:pb + 2])
            # k-sum (sel_neg count) into its own slot, row0 only
            dve.tensor_copy(PART[0:1, 16 + b:17 + b], KS[0:1, :])

        # ---------- final partition reduction ----------
        fin = pps.tile([18, 1], F32, tag="fin", name="fin")
        nc.tensor.matmul(fin[:], PART[:], ONESC[:], start=True, stop=True)
        OUTT = pfin.tile([18, 1], F32, tag="outt", name="outt")
        act.activation(OUTT[:], fin[:], Act.Copy)
        # PART[0, pb+7] and PART[1, pb+7] were row-local values; the matmul
        # summed over partitions, so they came through unscaled. OK.
        nc.sync.dma_start(aps["out"], OUTT[:])


_CACHE = {}


def _get_compiled():
    if "nc" in _CACHE:
        return _CACHE["nc"]
    nc = bacc.Bacc("TRN2", target_bir_lowering=False, debug=False)
    aps = {
        "pred0": nc.dram_tensor("pred0", [SPC, 24, 128, 128], F32,
                                kind="ExternalInput").ap(),
        "pred1": nc.dram_tensor("pred1", [SPC, 24, 64, 64], F32,
                                kind="ExternalInput").ap(),
        "pred2": nc.dram_tensor("pred2", [SPC, 24, 32, 32], F32,
                                kind="ExternalInput").ap(),
        "predi0": nc.dram_tensor("predi0", [P * NANCH, NF], F32,
                                 kind="ExternalInput").ap(),
        "predi1": nc.dram_tensor("predi1", [P * NANCH, NF], F32,
                                 kind="ExternalInput").ap(),
        "boxc": nc.dram_tensor("boxc", [SPC, 120, 4], F32,
                               kind="ExternalInput").ap(),
        "sc3": nc.dram_tensor("sc3", [SPC, 120, 3], F32,
                              kind="ExternalInput").ap(),
        "bbrow": nc.dram_tensor("bbrow", [SPC, 1, 200], F32,
                                kind="ExternalInput").ap(),
        "boxt0": nc.dram_tensor("boxt0", [NBOX, 8], F32,
                                kind="ExternalInput").ap(),
        "boxt1": nc.dram_tensor("boxt1", [NBOX, 8], F32,
                                kind="ExternalInput").ap(),
        "xl3": nc.dram_tensor("xl3", [120, 224], F32,
                              kind="ExternalInput").ap(),
        "xh3": nc.dram_tensor("xh3", [120, 224], F32,
                              kind="ExternalInput").ap(),
        "yl3": nc.dram_tensor("yl3", [120, 224], F32,
                              kind="ExternalInput").ap(),
        "yh3": nc.dram_tensor("yh3", [120, 224], F32,
                              kind="ExternalInput").ap(),
        "msk3": nc.dram_tensor("msk3", [120, 3], F16,
                               kind="ExternalInput").ap(),
        "out": nc.dram_tensor("out", [18, 1], F32,
                              kind="ExternalOutput").ap(),
    }
    with tile.TileContext(nc) as tc:
        _build_body(tc, aps)
    nc.compile()
    _CACHE["nc"] = nc
    return nc


def _host_geometry(anchors0, anchors1, anchors2):
    """Extract per-axis marginals from the grid-structured anchors."""
    HW = [(128, 128), (64, 64), (32, 32)]
    ancs = [np.asarray(anchors0, np.float32),
            np.asarray(anchors1, np.float32),
            np.asarray(anchors2, np.float32)]
    xl, xh, yl, yh, acx, wa, ha = [], [], [], [], [], [], []
    for (H, W), anc in zip(HW, ancs):
        arr = anc.reshape(H, W, 3, 4)
        xl.append(arr[0, :, :, 0].T.copy())   # [3, W]
        xh.append(arr[0, :, :, 2].T.copy())
        yl.append(arr[:, 0, :, 1].T.copy())   # [3, H]
        yh.append(arr[:, 0, :, 3].T.copy())
        wa.append(xh[-1][:, 0] - xl[-1][:, 0])        # [3]
        ha.append(yh[-1][:, 0] - yl[-1][:, 0])
        acx.append((xl[-1] + xh[-1]) * 0.5)
    return xl, xh, yl, yh, acx, wa, ha


def _prep_inputs(pred0, pred1, pred2, anchors0, anchors1, anchors2,
                 boxes, labels):
    B = pred0.shape[0]
    xl, xh, yl, yh, acx, wa, ha = _host_geometry(anchors0, anchors1,
                                                 anchors2)
    area9 = np.array([wa[s] * ha[s] for s in range(3)], np.float32)  # [3,3]

    # [3, 224] concat over scales then tile -> [120, 224]
    def cat3(v):
        return np.concatenate([v[0], v[1], v[2]], axis=1)  # [3, 224]

    # rows ordered a-major: row = a * NBOX + j
    xl3 = np.repeat(cat3(xl), NBOX, axis=0).astype(np.float32)
    xh3 = np.repeat(cat3(xh), NBOX, axis=0).astype(np.float32)
    yl3 = np.repeat(cat3(yl), NBOX, axis=0).astype(np.float32)
    yh3 = np.repeat(cat3(yh), NBOX, axis=0).astype(np.float32)
    msk3 = np.repeat(np.eye(3, dtype=np.float16), NBOX, axis=0)  # [120, 3]

    boxes = np.asarray(boxes, np.float32)
    labels = np.asarray(labels)
    bx1, by1, bx2, by2 = (boxes[..., 0], boxes[..., 1], boxes[..., 2],
                          boxes[..., 3])
    bw = bx2 - bx1
    bh = by2 - by1
    barea = bw * bh + 1e-9
    bcx = bx1 + 0.5 * bw
    bcy = by1 + 0.5 * bh
    lnwb = np.log(bw)
    lnhb = np.log(bh)

    boxc = np.zeros((B, 120, 4), np.float32)
    sc3 = np.zeros((B, 120, 3), np.float32)
    bbrow = np.zeros((B, 1, 200), np.float32)
    boxt = np.zeros((B, NBOX, 8), np.float32)
    for bi in range(B):
        for a in range(3):
            pr = a * NBOX + np.arange(NBOX)
            boxc[bi, pr, 0] = bx1[bi]
            boxc[bi, pr, 1] = bx2[bi]
            boxc[bi, pr, 2] = by1[bi]
            boxc[bi, pr, 3] = by2[bi]
            for s in range(3):
                sc3[bi, pr, s] = QSC / (area9[s, a] + barea[bi])
        bbrow[bi, 0, 0:40] = bx1[bi]
        bbrow[bi, 0, 40:80] = bx2[bi]
        bbrow[bi, 0, 80:120] = by1[bi]
        bbrow[bi, 0, 120:160] = by2[bi]
        bbrow[bi, 0, 160:200] = barea[bi]
        boxt[bi, :, 0] = bcx[bi]
        boxt[bi, :, 1] = bcy[bi]
        boxt[bi, :, 2] = lnwb[bi]
        boxt[bi, :, 3] = lnhb[bi]
        boxt[bi, :, 4] = labels[bi].astype(np.float32)

    # PREDI [B, 128*672, NF]: row p*672+col
    # fields: 0-3 deltas, 4-6 cls, 7 xl, 8 xh, 9 yl, 10 yh, 11 acx,
    #         12 rwa, 13 rha, 14 lnwa, 15 lnha, 16 A
    predi = np.zeros((B, P, NANCH, NF), np.float32)
    preds = [np.asarray(pred0, np.float32), np.asarray(pred1, np.float32),
             np.asarray(pred2, np.float32)]
    for s, (W, Hs, co, cw, yo, ywd) in enumerate(SC):
        pr = preds[s].reshape(B, 3, 8, Hs, W)
        blk = np.transpose(pr, (0, 3, 1, 4, 2))  # [B, y, a, x, ch]
        # deltas 0-3 -> fields 0-3; cls 5-7 -> fields 4-6
        predi[:, 0:Hs, co:co + cw, 0:4] = \
            blk[..., 0:4].reshape(B, Hs, 3 * W, 4)
        predi[:, 0:Hs, co:co + cw, 4:7] = \
            blk[..., 5:8].reshape(B, Hs, 3 * W, 3)
        for a in range(3):
            c0, c1 = co + a * W, co + (a + 1) * W
            predi[:, :, c0:c1, 7] = xl[s][a][None, None, :]
            predi[:, :, c0:c1, 8] = xh[s][a][None, None, :]
            predi[:, 0:Hs, c0:c1, 9] = yl[s][a][None, :, None]
            predi[:, 0:Hs, c0:c1, 10] = yh[s][a][None, :, None]
            predi[:, :, c0:c1, 11] = acx[s][a][None, None, :]
            predi[:, :, c0:c1, 12] = 1.0 / wa[s][a]
            predi[:, :, c0:c1, 13] = 1.0 / ha[s][a]
            predi[:, :, c0:c1, 14] = np.log(wa[s][a])
            predi[:, :, c0:c1, 15] = np.log(ha[s][a])
            predi[:, :, c0:c1, 16] = area9[s, a]
    predi = predi.reshape(B, P * NANCH, NF)

    return dict(xl3=xl3, xh3=xh3, yl3=yl3, yh3=yh3, msk3=msk3,
                boxc=boxc, sc3=sc3, bbrow=bbrow, boxt=boxt, predi=predi)


def kernel(pred0, pred1, pred2, anchors0, anchors1, anchors2, boxes,
           labels, _want_results=False, _trace=False):
    nc = _get_compiled()
    hp = _prep_inputs(pred0, pred1, pred2, anchors0, anchors1, anchors2,
                      boxes, labels)
    in_maps = []
    for c in range(NCORES):
        sl = slice(c * SPC, (c + 1) * SPC)
        in_maps.append({
            "pred0": np.ascontiguousarray(pred0[sl], np.float32),
            "pred1": np.ascontiguousarray(pred1[sl], np.float32),
            "pred2": np.ascontiguousarray(pred2[sl], np.float32),
            "predi0": np.ascontiguousarray(hp["predi"][c * SPC]),
            "predi1": np.ascontiguousarray(hp["predi"][c * SPC + 1]),
            "boxc": np.ascontiguousarray(hp["boxc"][sl]),
            "sc3": np.ascontiguousarray(hp["sc3"][sl]),
            "bbrow": np.ascontiguousarray(hp["bbrow"][sl]),
            "boxt0": np.ascontiguousarray(hp["boxt"][c * SPC]),
            "boxt1": np.ascontiguousarray(hp["boxt"][c * SPC + 1]),
            "xl3": hp["xl3"], "xh3": hp["xh3"],
            "yl3": hp["yl3"], "yh3": hp["yh3"], "msk3": hp["msk3"],
        })
    res = bass_utils.run_bass_kernel_spmd(
        nc, in_maps, core_ids=list(range(NCORES)), trace=_trace)
    parts = np.stack([res.results[c]["out"][:, 0] for c in range(NCORES)])
    tot = parts.sum(axis=0, dtype=np.float64)
    tot_obj = tot_cls = tot_loc = tot_pos = tot_neg = 0.0
    for b in range(SPC):
        pb = b * 8
        tot_obj += tot[pb + 0] + tot[pb + 6] + tot[pb + 7]
        tot_cls += tot[pb + 1]
        tot_loc += tot[pb + 2]
        tot_pos += tot[pb + 3] + tot[pb + 4] + tot[pb + 5]
        tot_neg += tot[16 + b]
    norm = np.float32(max(tot_pos, 1.0))
    lo = np.float32(tot_obj / norm)
    lc = np.float32(tot_cls / norm)
    ll = np.float32(tot_loc / norm)
    ltot = np.float32(lo + lc + np.float32(2.0) * ll)
    out = (lo, lc, ll, ltot, np.float32(tot_pos), np.float32(tot_neg))
    out = tuple(np.asarray(v, np.float32) for v in out)
    if _want_results:
        return out, res
    return out


# revision 16
# speedup vs baseline: 1.1534x; 1.1534x over previous
"""Trainium2 Bass kernel v2 for the 3-scale anchor DetectionLoss.

Data-parallel over batch: 16 samples -> 8 cores x 2 samples. Host sums
the per-core partial accumulators and applies the global normalizer.

Key structure (per sample):
- Anchor layout [128p, 672]: partition = grid row y; cols = s0 (3 sizes x
  128 x), s1 (3 x 64, rows 0..63), s2 (3 x 32, rows 0..31).
- IOU surrogate q = 64 * inter / (A + B): monotone in IOU, so argmax and
  the pos/neg thresholds (iou>=.5 <=> q>=64/3; iou<.3 <=> q<192/13)
  transfer. inter is separable: inter = fy(y) * fx(x), so the per-box
  pair stage is ONE rank-3 outer-product matmul per scale on the PE
  (lhsT = fy rows, rhs = block-diag fx pre-scaled by 64/(A+B)).
- ACT evacuates PSUM->fp16; DVE keeps a running max (BEST). pos/neg come
  from BEST; invalid (ragged) rows get BEST init 16.0, between the two
  scaled thresholds, so they are neither pos nor neg.
- Hard-negative mining: per-scale binary search for the k-th largest
  masked objectness loss; exact top-k sum via S(>thr) + (k-cnt)*thr.
- cls/loc losses only touch positives: per-partition top-16 positive
  columns are extracted with max8/match_replace/max_index, their data
  gathered via indirect DMA from host-interleaved DRAM tables, the
  matched box found by recomputing the 40-box q-strip per slot, and the
  small [128,16] tiles carry the SmoothL1 + CE math.
"""

import numpy as np
from contextlib import ExitStack

import concourse.bass as bass
import concourse.tile as tile
from concourse import bacc, mybir
from concourse import bass_utils
from concourse import bass_isa

F32 = mybir.dt.float32
F16 = mybir.dt.float16
U8 = mybir.dt.uint8
U16 = mybir.dt.uint16
U32 = mybir.dt.uint32
I32 = mybir.dt.int32
Alu = mybir.AluOpType
Act = mybir.ActivationFunctionType
Ax = mybir.AxisListType

NCORES = 8
SPC = 2
NBOX = 40
P = 128
NSLOT = 16
NITER = 9
QSC = 64.0                  # q scale to keep 1/(A+B) in fp16 normal range
POS_THR = QSC / 3.0         # q >= this  <=> iou >= 0.5
NEG_THR = QSC * 0.3 / 1.3   # q <  this  <=> iou < 0.3
GARB = 16.0                 # between NEG_THR (14.77) and POS_THR (21.33)

# scale: (W, H, fxd col off, fxd width, y-block off, y-width)
SC = [(128, 128, 0, 384, 0, 128), (64, 64, 384, 192, 128, 64),
      (32, 32, 576, 96, 192, 32)]
NANCH = 672                 # anchor cols per sample tile
NF = 20                     # fields per PREDI row
# PREDI fields: 0-3 deltas, 4-6 cls logits, 7 xl, 8 xh, 9 yl, 10 yh,
# 11 acx, 12 rwa, 13 rha, 14 lnwa, 15 lnha, 16 A


def bc_ins(ap, dim, n):
    """Insert a stride-0 dim of size n at position dim."""
    layout = [list(d) for d in ap.ap]
    layout.insert(dim, [0, n])
    return bass.AP(ap.tensor, ap.offset, layout)


def _build_body(tc, aps):
    nc = tc.nc
    dve = nc.vector
    act = nc.scalar
    gp = nc.gpsimd

    with ExitStack() as ctx:
        pc = ctx.enter_context(tc.tile_pool(name="const", bufs=1))
        pp = ctx.enter_context(tc.tile_pool(name="prep", bufs=2))
        pq = ctx.enter_context(tc.tile_pool(name="qpair", bufs=1))
        pt = ctx.enter_context(tc.tile_pool(name="ptrans", bufs=1))
        pqs = ctx.enter_context(tc.tile_pool(name="qpsum", bufs=1,
                                             space="PSUM"))
        pd = ctx.enter_context(tc.tile_pool(name="dense", bufs=2))
        psl = ctx.enter_context(tc.tile_pool(name="slots", bufs=1))
        pmi = ctx.enter_context(tc.tile_pool(name="mine", bufs=2))
        pfin = ctx.enter_context(tc.tile_pool(name="fin", bufs=1))
        pps = ctx.enter_context(tc.tile_pool(name="smallps", bufs=1,
                                             space="PSUM"))

        # ---------- per-kernel constants ----------
        XL3 = pc.tile([120, 224], F32, tag="xl3", name="xl3")
        XH3 = pc.tile([120, 224], F32, tag="xh3", name="xh3")
        YL3 = pc.tile([120, 224], F32, tag="yl3", name="yl3")
        YH3 = pc.tile([120, 224], F32, tag="yh3", name="yh3")
        MSK3 = pc.tile([120, 3], F16, tag="msk3", name="msk3")
        for t, k in ((XL3, "xl3"), (XH3, "xh3"), (YL3, "yl3"),
                     (YH3, "yh3"), (MSK3, "msk3")):
            nc.sync.dma_start(t[:], aps[k])

        ONESC = pc.tile([P, 1], F32, tag="onesc", name="onesc")
        dve.memset(ONESC[:], 1.0)
        ONESR = pc.tile([1, P], F32, tag="onesr", name="onesr")
        dve.memset(ONESR[:], 1.0)

        # key weights 1 - c/2048 (descending, distinct, fp16-exact)
        KEYW = pc.tile([P, NANCH], F16, tag="keyw", name="keyw")
        JIF = pc.tile([P, NSLOT * NBOX], F32, tag="jif", name="jif")
        POF = pc.tile([P, 1], F32, tag="pof", name="pof")
        with tc.tile_pool(name="initscr", bufs=1) as pin:
            kwi = pin.tile([P, NANCH], I32, tag="kwi", name="kwi")
            gp.iota(kwi[:], [[1, NANCH]], base=0, channel_multiplier=0)
            kwf = pin.tile([P, NANCH], F32, tag="kwf", name="kwf")
            dve.tensor_copy(kwf[:], kwi[:])
            dve.tensor_scalar(KEYW[:], kwf[:], -1.0 / 2048.0, 1.0,
                              Alu.mult, Alu.add)
            ji = pin.tile([P, NSLOT * NBOX], I32, tag="ji", name="ji")
            gp.iota(ji[:], [[0, NSLOT], [1, NBOX]], base=0,
                    channel_multiplier=0)
            dve.tensor_copy(JIF[:], ji[:])
            pofi = pin.tile([P, 1], I32, tag="pofi", name="pofi")
            gp.iota(pofi[:], [[1, 1]], base=0, channel_multiplier=NANCH)
            dve.tensor_copy(POF[:], pofi[:])      # p * 672

        PART = pfin.tile([P, 18], F32, tag="part", name="part")
        dve.memset(PART[:], 0.0)

        for b in range(SPC):
            pb = b * 8   # PART col base: 0:objpos 1:cls 2:loc 3-5:npos3
            #              6(row0 only):k3x3? -> use cols 6,7 specially

            # ---------- box-dependent prep ----------
            BOXC = pp.tile([120, 4], F32, tag="boxc", name=f"boxc{b}")
            nc.sync.dma_start(BOXC[:], aps["boxc"][b])
            SC3 = pp.tile([120, 3], F32, tag="sc3", name=f"sc3{b}")
            nc.sync.dma_start(SC3[:], aps["sc3"][b])

            # fy per (j,a): [120, 224] fp16
            t1 = pp.tile([120, 224], F32, tag="t1", name=f"t1{b}")
            t2 = pp.tile([120, 224], F32, tag="t2", name=f"t2{b}")
            FYJ = pp.tile([120, 224], F16, tag="fyj", name=f"fyj{b}")
            dve.tensor_scalar(t1[:], YH3[:], BOXC[:, 3:4], None, Alu.min)
            dve.tensor_scalar(t2[:], YL3[:], BOXC[:, 2:3], None, Alu.max)
            dve.tensor_tensor(t1[:], t1[:], t2[:], Alu.subtract)
            act.activation(FYJ[:], t1[:], Act.Relu)
            # fx, scaled by 64/(A+B) per scale block
            fx1 = pp.tile([120, 224], F32, tag="fx1", name=f"fx1{b}")
            fx2 = pp.tile([120, 224], F32, tag="fx2", name=f"fx2{b}")
            FXS = pp.tile([120, 224], F16, tag="fxs", name=f"fxs{b}")
            dve.tensor_scalar(fx1[:], XH3[:], BOXC[:, 1:2], None, Alu.min)
            dve.tensor_scalar(fx2[:], XL3[:], BOXC[:, 0:1], None, Alu.max)
            dve.tensor_tensor(fx1[:], fx1[:], fx2[:], Alu.subtract)
            xo = 0
            for s, (W, H, co, cw, yo, yw) in enumerate(SC):
                act.activation(FXS[:, xo:xo + W], fx1[:, xo:xo + W],
                               Act.Relu, scale=SC3[:, s:s + 1])
                xo += W
            # block-diag expand: FXD[p, (a', x)] = FXS[p, x] * MSK3[p, a']
            FXD = pp.tile([120, NANCH], F16, tag="fxd", name=f"fxd{b}")
            xo = 0
            for s, (W, H, co, cw, yo, yw) in enumerate(SC):
                src = bc_ins(FXS[:, xo:xo + W], 1, 3)
                msk = MSK3[:].to_broadcast([120, 3, W])
                dve.tensor_tensor(
                    FXD[:, co:co + cw].rearrange("p (a x) -> p a x", a=3),
                    src, msk, Alu.mult)
                xo += W

            # ---------- dense obj logits ----------
            POBJ = pd.tile([P, NANCH], F32, tag="pobj", name=f"pobj{b}")
            dve.memset(POBJ[64:128, 384:576], 0.0)
            dve.memset(POBJ[32:64, 576:672], 0.0)
            dve.memset(POBJ[64:128, 576:672], 0.0)
            preds = [aps["pred0"], aps["pred1"], aps["pred2"]]
            for s, (W, H, co, cw, yo, yw) in enumerate(SC):
                for a in range(3):
                    nc.sync.dma_start(
                        POBJ[0:H, co + a * W: co + (a + 1) * W],
                        preds[s][b, a * 8 + 4])

            # ---------- pair stage ----------
            BESTe = pd.tile([P, NANCH], F16, tag="beste", name=f"beste{b}")
            BESTo = pd.tile([P, NANCH], F16, tag="besto", name=f"besto{b}")
            for t in (BESTe, BESTo):
                dve.memset(t[:, 0:384], 0.0)
                dve.memset(t[0:64, 384:576], 0.0)
                dve.memset(t[0:32, 576:672], 0.0)
                dve.memset(t[64:128, 384:576], GARB)
                dve.memset(t[32:64, 576:672], GARB)
                dve.memset(t[64:128, 576:672], GARB)

            # PE operands must sit at base partition 0: reshuffle the
            # [120, *] (j,a)-row tiles into [3, boxes*cols] chunks via DMA.
            CB = 10                      # boxes per chunk
            for ch in range(NBOX // CB):
                FYT = pt.tile([3, CB * 224], F16, tag=f"fyt{ch % 2}",
                              name=f"fyt{b}_{ch}")
                FXT = pt.tile([3, CB * NANCH], F16, tag=f"fxt{ch % 2}",
                              name=f"fxt{b}_{ch}")
                for a in range(3):
                    rows = slice(40 * a + CB * ch, 40 * a + CB * (ch + 1))
                    nc.sync.dma_start(FYT[a:a + 1, :], FYJ[rows, :])
                    nc.sync.dma_start(FXT[a:a + 1, :], FXD[rows, :])
                for jj in range(CB):
                    j = ch * CB + jj
                    yo = jj * 224
                    xo = jj * NANCH
                    psA = pqs.tile([P, 384], F32, tag=f"psA{j % 3}",
                                   name=f"psA{b}_{j}")
                    psB = pqs.tile([64, 288], F32, tag=f"psB{j % 3}",
                                   name=f"psB{b}_{j}")
                    nc.tensor.matmul(psA[:], FYT[0:3, yo:yo + 128],
                                     FXT[0:3, xo:xo + 384],
                                     start=True, stop=True)
                    nc.tensor.matmul(psB[0:64, 0:192],
                                     FYT[0:3, yo + 128:yo + 192],
                                     FXT[0:3, xo + 384:xo + 576],
                                     start=True, stop=True)
                    nc.tensor.matmul(psB[0:32, 192:288],
                                     FYT[0:3, yo + 192:yo + 224],
                                     FXT[0:3, xo + 576:xo + 672],
                                     start=True, stop=True)
                    QA = pq.tile([P, 384], F16, tag=f"qa{j % 3}",
                                 name=f"qa{b}_{j}")
                    QB = pq.tile([64, 288], F16, tag=f"qb{j % 3}",
                                 name=f"qb{b}_{j}")
                    act.activation(QA[:], psA[:], Act.Copy)
                    act.activation(QB[0:64, 0:192], psB[0:64, 0:192],
                                   Act.Copy)
                    act.activation(QB[0:32, 192:288], psB[0:32, 192:288],
                                   Act.Copy)
                    acc = BESTe if j % 2 == 0 else BESTo
                    dve.tensor_tensor(acc[:, 0:384], acc[:, 0:384], QA[:],
                                      Alu.max)
                    dve.tensor_tensor(acc[0:64, 384:576],
                                      acc[0:64, 384:576],
                                      QB[0:64, 0:192], Alu.max)
                    dve.tensor_tensor(acc[0:32, 576:672],
                                      acc[0:32, 576:672],
                                      QB[0:32, 192:288], Alu.max)
            BEST = pd.tile([P, NANCH], F16, tag="best", name=f"best{b}")
            dve.tensor_tensor(BEST[:], BESTe[:], BESTo[:], Alu.max)

            # ---------- masks / dense losses ----------
            POS = pd.tile([P, NANCH], F16, tag="pos", name=f"pos{b}")
            NEG = pd.tile([P, NANCH], F16, tag="neg", name=f"neg{b}")
            dve.tensor_scalar(POS[:], BEST[:], POS_THR, None, Alu.is_ge)
            dve.tensor_scalar(NEG[:], BEST[:], NEG_THR, None, Alu.is_lt)

            # softplus(x) = relu(x) + ln(1 + exp(-|x|)), in-place chains
            AX = pd.tile([P, NANCH], F32, tag="ax", name=f"ax{b}")
            SP = pd.tile([P, NANCH], F32, tag="sp", name=f"sp{b}")
            act.activation(AX[:], POBJ[:], Act.Abs)
            act.activation(AX[:], AX[:], Act.Exp, scale=-1.0)
            act.activation(AX[:], AX[:], Act.Ln, bias=1.0)
            act.activation(SP[:], POBJ[:], Act.Relu)
            dve.tensor_tensor(SP[:], SP[:], AX[:], Alu.add)
            # obj positive part: sum pos * (SP - x); POBJ becomes (SP - x)
            dve.tensor_tensor(POBJ[:], SP[:], POBJ[:], Alu.subtract)
            scr = pd.tile([P, NANCH], F32, tag="scr", name=f"scr{b}")
            dve.tensor_tensor(scr[:], POBJ[:], POS[:], Alu.mult)
            dve.tensor_scalar(AX[:], scr[:], 0.0, 0.0, Alu.add, Alu.add,
                              accum_out=PART[:, pb:pb + 1])
            # NEGL = NEG * SP (fp16)
            NEGL = pd.tile([P, NANCH], F16, tag="negl", name=f"negl{b}")
            dve.tensor_tensor(NEGL[:], NEG[:], SP[:], Alu.mult)

            # per-scale pos/neg counts
            mc16a = pd.tile([P, NANCH], F16, tag="mc16a", name=f"mc16a{b}")
            CNT = pmi.tile([P, 8], F32, tag="cnt", name=f"cnt{b}")
            for s, (W, H, co, cw, yo, yw) in enumerate(SC):
                blk = slice(co, co + cw)
                dve.tensor_scalar(mc16a[:, blk], POS[:, blk], 0.0, 0.0,
                                  Alu.add, Alu.add,
                                  accum_out=CNT[:, s:s + 1])
                dve.tensor_scalar(mc16a[:, blk], NEG[:, blk], 0.0, 0.0,
                                  Alu.add, Alu.add,
                                  accum_out=CNT[:, 4 + s:5 + s])
            NPOS3 = pmi.tile([P, 3], F32, tag="npos3", name=f"npos3{b}")
            NNEG3 = pmi.tile([P, 3], F32, tag="nneg3", name=f"nneg3{b}")
            gp.partition_all_reduce(NPOS3[:], CNT[:, 0:3], P,
                                    bass_isa.ReduceOp.add)
            gp.partition_all_reduce(NNEG3[:], CNT[:, 4:7], P,
                                    bass_isa.ReduceOp.add)
            dve.tensor_copy(PART[0:1, pb + 3:pb + 6], NPOS3[0:1, :])

            # ---------- mining: per-scale k-th threshold ----------
            K3 = pmi.tile([P, 3], F32, tag="k3", name=f"k3{b}")
            dve.tensor_scalar(K3[:], NPOS3[:], 1.0, 3.0, Alu.max, Alu.mult)
            dve.tensor_tensor(K3[:], K3[:], NNEG3[:], Alu.min)
            HI3 = pmi.tile([P, 3], F32, tag="hi3", name=f"hi3{b}")
            LO3 = pmi.tile([P, 3], F32, tag="lo3", name=f"lo3{b}")
            MID3 = pmi.tile([P, 3], F32, tag="mid3", name=f"mid3{b}")
            CP3 = pmi.tile([P, 3], F32, tag="cp3", name=f"cp3{b}")
            CT3 = pmi.tile([P, 3], F32, tag="ct3", name=f"ct3{b}")
            GTK = pmi.tile([P, 3], U8, tag="gtk", name=f"gtk{b}")
            LEK = pmi.tile([P, 3], U8, tag="lek", name=f"lek{b}")
            RM3 = pmi.tile([P, 3], F32, tag="rm3", name=f"rm3{b}")
            for s, (W, H, co, cw, yo, yw) in enumerate(SC):
                dve.tensor_reduce(RM3[:, s:s + 1], NEGL[:, co:co + cw],
                                  Ax.X, Alu.max)
            gp.partition_all_reduce(HI3[:], RM3[:], P,
                                    bass_isa.ReduceOp.max)
            dve.memset(LO3[:], 0.0)
            mc16 = pd.tile([P, NANCH], F16, tag="mc16", name=f"mc16{b}")
            for it in range(NITER):
                dve.tensor_tensor(MID3[:], LO3[:], HI3[:], Alu.add)
                dve.tensor_scalar(MID3[:], MID3[:], 0.5, None, Alu.mult)
                for s, (W, H, co, cw, yo, yw) in enumerate(SC):
                    blk = slice(co, co + cw)
                    dve.tensor_scalar(mc16[:, blk], NEGL[:, blk],
                                      MID3[:, s:s + 1], 0.0, Alu.is_gt,
                                      Alu.add, accum_out=CP3[:, s:s + 1])
                gp.partition_all_reduce(CT3[:], CP3[:], P,
                                        bass_isa.ReduceOp.add)
                dve.tensor_tensor(GTK[:], CT3[:], K3[:], Alu.is_gt)
                dve.tensor_tensor(LEK[:], CT3[:], K3[:], Alu.is_le)
                dve.copy_predicated(LO3[:], GTK[:], MID3[:])
                dve.copy_predicated(HI3[:], LEK[:], MID3[:])
            # top-k sum = S(>thr) + (k - cnt(>thr)) * thr ; thr = HI3
            SG3 = pmi.tile([P, 3], F32, tag="sg3", name=f"sg3{b}")
            for s, (W, H, co, cw, yo, yw) in enumerate(SC):
                blk = slice(co, co + cw)
                dve.tensor_scalar(mc16[:, blk], NEGL[:, blk],
                                  HI3[:, s:s + 1], 0.0, Alu.is_gt,
                                  Alu.add, accum_out=CP3[:, s:s + 1])
                dve.tensor_tensor(mc16[:, blk], NEGL[:, blk], mc16[:, blk],
                                  Alu.mult)
                dve.tensor_scalar(mc16[:, blk], mc16[:, blk], 0.0, 0.0,
                                  Alu.add, Alu.add,
                                  accum_out=SG3[:, s:s + 1])
            gp.partition_all_reduce(CT3[:], CP3[:], P,
                                    bass_isa.ReduceOp.add)
            # per-partition SG3 partials summed via PART (full column)
            dve.tensor_copy(PART[:, pb + 6:pb + 7],
                            SG3[:, 0:1])
            dve.tensor_tensor(PART[:, pb + 6:pb + 7], PART[:, pb + 6:pb + 7],
                              SG3[:, 1:2], Alu.add)
            dve.tensor_tensor(PART[:, pb + 6:pb + 7], PART[:, pb + 6:pb + 7],
                              SG3[:, 2:3], Alu.add)
            # (k - cnt) * thr + k  -> row0 only (bcast-identical values)
            TK = pmi.tile([P, 3], F32, tag="tk", name=f"tk{b}")
            dve.tensor_tensor(TK[:], K3[:], CT3[:], Alu.subtract)
            dve.tensor_tensor(TK[:], TK[:], HI3[:], Alu.mult)
            dve.tensor_copy(PART[0:1, pb + 7:pb + 8], TK[0:1, 0:1])
            dve.tensor_tensor(PART[0:1, pb + 7:pb + 8],
                              PART[0:1, pb + 7:pb + 8], TK[0:1, 1:2], Alu.add)
            dve.tensor_tensor(PART[0:1, pb + 7:pb + 8],
                              PART[0:1, pb + 7:pb + 8], TK[0:1, 2:3], Alu.add)
            KS = pmi.tile([P, 1], F32, tag="ks", name=f"ks{b}")
            dve.tensor_copy(KS[:], K3[:, 0:1])
            dve.tensor_tensor(KS[:], KS[:], K3[:, 1:2], Alu.add)
            dve.tensor_tensor(KS[:], KS[:], K3[:, 2:3], Alu.add)

            # ---------- positive slots ----------
            KEY = psl.tile([P, NANCH], F16, tag="key", name=f"key{b}")
            dve.tensor_tensor(KEY[:], POS[:], KEYW[:], Alu.mult)
            K8a = psl.tile([P, 8], F16, tag="k8a", name=f"k8a{b}")
            K8b = psl.tile([P, 8], F16, tag="k8b", name=f"k8b{b}")
            IX8a = psl.tile([P, 8], U16, tag="ix8a", name=f"ix8a{b}")
            IX8b = psl.tile([P, 8], U16, tag="ix8b", name=f"ix8b{b}")
            KEY2 = psl.tile([P, NANCH], F16, tag="key2", name=f"key2{b}")
            dve.max(K8a[:], KEY[:])
            dve.max_index(IX8a[:], K8a[:], KEY[:])
            dve.match_replace(KEY2[:], K8a[:], KEY[:], -1.0)
            dve.max(K8b[:], KEY2[:])
            dve.max_index(IX8b[:], K8b[:], KEY2[:])
            VAL = psl.tile([P, NSLOT], F16, tag="val", name=f"val{b}")
            dve.tensor_scalar(VAL[:, 0:8], K8a[:], 0.0, None, Alu.is_gt)
            dve.tensor_scalar(VAL[:, 8:16], K8b[:], 0.0, None, Alu.is_gt)
            COLU = psl.tile([P, NSLOT], U32, tag="colu", name=f"colu{b}")
            dve.tensor_copy(COLU[:, 0:8], IX8a[:])
            dve.tensor_copy(COLU[:, 8:16], IX8b[:])
            COLF = psl.tile([P, NSLOT], F32, tag="colf", name=f"colf{b}")
            dve.tensor_copy(COLF[:], COLU[:])

            # gather PREDI rows (pred + geometry) at p*672+col, per slot
            OFFP = psl.tile([P, NSLOT], F32, tag="offp", name=f"offp{b}")
            dve.tensor_scalar(OFFP[:], COLF[:], POF[:, 0:1], 0.0, Alu.add,
                              Alu.add)
            OFFPU = psl.tile([P, NSLOT], U32, tag="offpu", name=f"offpu{b}")
            dve.tensor_copy(OFFPU[:], OFFP[:])
            GSA = psl.tile([P, NSLOT * NF], F32, tag="gsa", name=f"gsa{b}")
            for s in range(NSLOT):
                ofs = psl.tile([P, 1], U32, tag=f"ofs{s}", name=f"ofs{b}_{s}")
                dve.tensor_copy(ofs[:], OFFPU[:, s:s + 1])
                gp.indirect_dma_start(
                    out=GSA[:, s * NF:(s + 1) * NF], out_offset=None,
                    in_=aps[f"predi{b}"][:],
                    in_offset=bass.IndirectOffsetOnAxis(ap=ofs[:], axis=0))
            # transpose (slot, field) -> (field, slot) in one strided copy
            GT = psl.tile([P, NF * NSLOT], F32, tag="gt", name=f"gt{b}")
            dve.tensor_copy(GT[:].rearrange("p (f s) -> p s f", s=NSLOT),
                            GSA[:].rearrange("p (s f) -> p s f", f=NF))

            def fld(fi, name):
                return GT[:, fi * NSLOT:(fi + 1) * NSLOT]

            # strip inputs
            XLs = fld(7, "xls")
            XHs = fld(8, "xhs")
            YLs = fld(9, "yls")
            YHs = fld(10, "yhs")
            AAs = fld(16, "aas")

            # box coord broadcast [128, 200] via PE (bx1 bx2 by1 by2 barea)
            bbp = pps.tile([P, 200], F32, tag="bbp", name=f"bbp{b}")
            BROW = psl.tile([1, 200], F32, tag="brow", name=f"brow{b}")
            nc.sync.dma_start(BROW[:], aps["bbrow"][b])
            nc.tensor.matmul(bbp[:], ONESR[:], BROW[:], start=True,
                             stop=True)
            BB = psl.tile([P, 200], F32, tag="bb", name=f"bb{b}")
            act.activation(BB[:], bbp[:], Act.Copy)

            # q strip [128, 16*40] fp32
            SJ = NSLOT * NBOX

            def strip_ov(name, lo_ap, hi_ap, blo, bhi):
                m1 = psl.tile([P, SJ], F32, tag=f"{name}1", name=f"{name}1{b}")
                m2 = psl.tile([P, SJ], F32, tag=f"{name}2", name=f"{name}2{b}")
                v3 = m1[:].rearrange("p (s j) -> p s j", j=NBOX)
                v4 = m2[:].rearrange("p (s j) -> p s j", j=NBOX)
                dve.tensor_tensor(v3, hi_ap, bhi, Alu.min)
                dve.tensor_tensor(v4, lo_ap, blo, Alu.max)
                dve.tensor_tensor(m1[:], m1[:], m2[:], Alu.subtract)
                r = psl.tile([P, SJ], F32, tag=f"{name}r", name=f"{name}r{b}")
                act.activation(r[:], m1[:], Act.Relu)
                return r

            xl_b = XLs.to_broadcast([P, NSLOT, NBOX])
            xh_b = XHs.to_broadcast([P, NSLOT, NBOX])
            yl_b = YLs.to_broadcast([P, NSLOT, NBOX])
            yh_b = YHs.to_broadcast([P, NSLOT, NBOX])
            bx1_b = bc_ins(BB[:, 0:40], 1, NSLOT)
            bx2_b = bc_ins(BB[:, 40:80], 1, NSLOT)
            by1_b = bc_ins(BB[:, 80:120], 1, NSLOT)
            by2_b = bc_ins(BB[:, 120:160], 1, NSLOT)
            FXP = strip_ov("fx", xl_b, xh_b, bx1_b, bx2_b)
            FYP = strip_ov("fy", yl_b, yh_b, by1_b, by2_b)
            # srec = QSC / (A + barea)
            ABJ = psl.tile([P, SJ], F32, tag="abj", name=f"abj{b}")
            dve.tensor_tensor(ABJ[:].rearrange("p (s j) -> p s j", j=NBOX),
                              AAs.to_broadcast([P, NSLOT, NBOX]),
                              bc_ins(BB[:, 160:200], 1, NSLOT), Alu.add)
            SRJ = psl.tile([P, SJ], F32, tag="srj", name=f"srj{b}")
            dve.reciprocal_approx_fast(SRJ[:], ABJ[:])
            QST = psl.tile([P, SJ], F32, tag="qst", name=f"qst{b}")
            dve.tensor_tensor(QST[:], FXP[:], FYP[:], Alu.mult)
            dve.tensor_tensor(QST[:], QST[:], SRJ[:], Alu.mult)
            # argmax-first over j
            BQ = psl.tile([P, NSLOT], F32, tag="bq", name=f"bq{b}")
            dve.tensor_reduce(BQ[:], QST[:].rearrange(
                "p (s j) -> p s j", j=NBOX), Ax.X, Alu.max)
            MSKJ = psl.tile([P, SJ], U8, tag="mskj", name=f"mskj{b}")
            dve.tensor_tensor(MSKJ[:].rearrange("p (s j) -> p s j", j=NBOX),
                              QST[:].rearrange("p (s j) -> p s j", j=NBOX),
                              BQ[:].to_broadcast([P, NSLOT, NBOX]),
                              Alu.is_ge)
            JM = psl.tile([P, SJ], F32, tag="jm", name=f"jm{b}")
            dve.memset(JM[:], 99.0)
            dve.copy_predicated(JM[:], MSKJ[:], JIF[:])
            JF = psl.tile([P, NSLOT], F32, tag="jf", name=f"jf{b}")
            dve.tensor_reduce(JF[:], JM[:].rearrange(
                "p (s j) -> p s j", j=NBOX), Ax.X, Alu.min)
            JU = psl.tile([P, NSLOT], U32, tag="ju", name=f"ju{b}")
            dve.tensor_copy(JU[:], JF[:])
            # gather matched box rows per slot
            BVA = psl.tile([P, NSLOT * 8], F32, tag="bva", name=f"bva{b}")
            for s in range(NSLOT):
                ofj = psl.tile([P, 1], U32, tag=f"ofj{s}", name=f"ofj{b}_{s}")
                dve.tensor_copy(ofj[:], JU[:, s:s + 1])
                gp.indirect_dma_start(
                    out=BVA[:, s * 8:(s + 1) * 8], out_offset=None,
                    in_=aps[f"boxt{b}"][:],
                    in_offset=bass.IndirectOffsetOnAxis(ap=ofj[:], axis=0))
            BVT = psl.tile([P, 8 * NSLOT], F32, tag="bvt", name=f"bvt{b}")
            dve.tensor_copy(BVT[:].rearrange("p (f s) -> p s f", s=NSLOT),
                            BVA[:].rearrange("p (s f) -> p s f", f=8))

            def bfld(fi, name):
                return BVT[:, fi * NSLOT:(fi + 1) * NSLOT]

            BCXs = bfld(0, "bcxs")
            BCYs = bfld(1, "bcys")
            LNWs = bfld(2, "lnws")
            LNHs = bfld(3, "lnhs")
            LABs = bfld(4, "labs")

            def st(name):
                return psl.tile([P, NSLOT], F32, tag=name, name=f"{name}{b}")

            def pfld(fi, name):
                return GT[:, fi * NSLOT:(fi + 1) * NSLOT]

            # ---------- loc loss on slots ----------
            ACXs = fld(11, "acxs")
            RWAs = fld(12, "rwas")
            RHAs = fld(13, "rhas")
            LNWAs = fld(14, "lnwas")
            LNHAs = fld(15, "lnhas")
            ACYs = st("acys")
            dve.tensor_tensor(ACYs[:], YLs, YHs, Alu.add)
            dve.tensor_scalar(ACYs[:], ACYs[:], 0.5, None, Alu.mult)
            encs = []
            e0 = st("e0")
            dve.tensor_tensor(e0[:], BCXs, ACXs, Alu.subtract)
            dve.tensor_tensor(e0[:], e0[:], RWAs, Alu.mult)
            encs.append(e0)
            e1 = st("e1")
            dve.tensor_tensor(e1[:], BCYs, ACYs[:], Alu.subtract)
            dve.tensor_tensor(e1[:], e1[:], RHAs, Alu.mult)
            encs.append(e1)
            e2 = st("e2")
            dve.tensor_tensor(e2[:], LNWs, LNWAs, Alu.subtract)
            encs.append(e2)
            e3 = st("e3")
            dve.tensor_tensor(e3[:], LNHs, LNHAs, Alu.subtract)
            encs.append(e3)
            SL = st("sl")
            first = True
            for c in range(4):
                pd_c = pfld(c, f"pd{c}")
                d = st(f"d{c}")
                dve.tensor_tensor(d[:], pd_c, encs[c][:], Alu.subtract)
                ad = st(f"ad{c}")
                act.activation(ad[:], d[:], Act.Abs)
                mm = st(f"mm{c}")
                dve.tensor_scalar(mm[:], ad[:], 1.0, None, Alu.min)
                q1 = st(f"q1{c}")
                dve.tensor_tensor(q1[:], mm[:], mm[:], Alu.mult)
                dve.tensor_scalar(q1[:], q1[:], 0.5, None, Alu.mult)
                u1 = st(f"u1{c}")
                dve.tensor_tensor(u1[:], ad[:], mm[:], Alu.subtract)
                dve.tensor_tensor(q1[:], q1[:], u1[:], Alu.add)
                if first:
                    dve.tensor_copy(SL[:], q1[:])
                    first = False
                else:
                    dve.tensor_tensor(SL[:], SL[:], q1[:], Alu.add)
            lscr = st("lscr")
            dve.scalar_tensor_tensor(lscr[:], SL[:], 0.0, VAL[:], Alu.add,
                                     Alu.mult,
                                     accum_out=PART[:, pb + 2:pb + 3])

            # ---------- cls loss on slots ----------
            c0 = pfld(4, "c0f")
            c1 = pfld(5, "c1f")
            c2 = pfld(6, "c2f")
            mx = st("mx")
            dve.tensor_tensor(mx[:], c0, c1, Alu.max)
            dve.tensor_tensor(mx[:], mx[:], c2, Alu.max)
            ssum = st("ssum")
            first = True
            for ci, cap in enumerate((c0, c1, c2)):
                dd = st(f"dd{ci}")
                dve.tensor_tensor(dd[:], cap, mx[:], Alu.subtract)
                ee = st(f"ee{ci}")
                act.activation(ee[:], dd[:], Act.Exp)
                if first:
                    dve.tensor_copy(ssum[:], ee[:])
                    first = False
                else:
                    dve.tensor_tensor(ssum[:], ssum[:], ee[:], Alu.add)
            lse = st("lse")
            act.activation(lse[:], ssum[:], Act.Ln)
            dve.tensor_tensor(lse[:], lse[:], mx[:], Alu.add)
            pick = st("pick")
            dve.tensor_copy(pick[:], c0)
            m1u = psl.tile([P, NSLOT], U8, tag="m1u", name=f"m1u{b}")
            m2u = psl.tile([P, NSLOT], U8, tag="m2u", name=f"m2u{b}")
            dve.tensor_scalar(m1u[:], LABs, 2.0, None, Alu.is_equal)
            dve.tensor_scalar(m2u[:], LABs, 3.0, None, Alu.is_equal)
            dve.copy_predicated(pick[:], m1u[:], c1)
            dve.copy_predicated(pick[:], m2u[:], c2)
            ce = st("ce")
            dve.tensor_tensor(ce[:], lse[:], pick[:], Alu.subtract)
            cscr = st("cscr")
            dve.scalar_tensor_tensor(cscr[:], ce[:], 0.0, VAL[:], Alu.add,
                                     Alu.mult,
                                     accum_out=PART[:, pb + 1:pb + 2])
            # k-sum (sel_neg count) into its own slot, row0 only
            dve.tensor_copy(PART[0:1, 16 + b:17 + b], KS[0:1, :])

        # ---------- final partition reduction ----------
        fin = pps.tile([18, 1], F32, tag="fin", name="fin")
        nc.tensor.matmul(fin[:], PART[:], ONESC[:], start=True, stop=True)
        OUTT = pfin.tile([18, 1], F32, tag="outt", name="outt")
        act.activation(OUTT[:], fin[:], Act.Copy)
        # PART[0, pb+7] and PART[1, pb+7] were row-local values; the matmul
        # summed over partitions, so they came through unscaled. OK.
        nc.sync.dma_start(aps["out"], OUTT[:])


_CACHE = {}


def _get_compiled():
    if "nc" in _CACHE:
        return _CACHE["nc"]
    nc = bacc.Bacc("TRN2", target_bir_lowering=False, debug=False)
    aps = {
        "pred0": nc.dram_tensor("pred0", [SPC, 24, 128, 128], F32,
                                kind="ExternalInput").ap(),
        "pred1": nc.dram_tensor("pred1", [SPC, 24, 64, 64], F32,
                                kind="ExternalInput").ap(),
        "pred2": nc.dram_tensor("pred2", [SPC, 24, 32, 32], F32,
                                kind="ExternalInput").ap(),
        "predi0": nc.dram_tensor("predi0", [P * NANCH, NF], F32,
                                 kind="ExternalInput").ap(),
        "predi1": nc.dram_tensor("predi1", [P * NANCH, NF], F32,
                                 kind="ExternalInput").ap(),
        "boxc": nc.dram_tensor("boxc", [SPC, 120, 4], F32,
                               kind="ExternalInput").ap(),
        "sc3": nc.dram_tensor("sc3", [SPC, 120, 3], F32,
                              kind="ExternalInput").ap(),
        "bbrow": nc.dram_tensor("bbrow", [SPC, 1, 200], F32,
                                kind="ExternalInput").ap(),
        "boxt0": nc.dram_tensor("boxt0", [NBOX, 8], F32,
                                kind="ExternalInput").ap(),
        "boxt1": nc.dram_tensor("boxt1", [NBOX, 8], F32,
                                kind="ExternalInput").ap(),
        "xl3": nc.dram_tensor("xl3", [120, 224], F32,
                              kind="ExternalInput").ap(),
        "xh3": nc.dram_tensor("xh3", [120, 224], F32,
                              kind="ExternalInput").ap(),
        "yl3": nc.dram_tensor("yl3", [120, 224], F32,
                              kind="ExternalInput").ap(),
        "yh3": nc.dram_tensor("yh3", [120, 224], F32,
                              kind="ExternalInput").ap(),
        "msk3": nc.dram_tensor("msk3", [120, 3], F16,
                               kind="ExternalInput").ap(),
        "out": nc.dram_tensor("out", [18, 1], F32,
                              kind="ExternalOutput").ap(),
    }
    with tile.TileContext(nc) as tc:
        _build_body(tc, aps)
    nc.compile()
    _CACHE["nc"] = nc
    return nc


def _host_geometry(anchors0, anchors1, anchors2):
    """Extract per-axis marginals from the grid-structured anchors."""
    HW = [(128, 128), (64, 64), (32, 32)]
    ancs = [np.asarray(anchors0, np.float32),
            np.asarray(anchors1, np.float32),
            np.asarray(anchors2, np.float32)]
    xl, xh, yl, yh, acx, wa, ha = [], [], [], [], [], [], []
    for (H, W), anc in zip(HW, ancs):
        arr = anc.reshape(H, W, 3, 4)
        xl.append(arr[0, :, :, 0].T.copy())   # [3, W]
        xh.append(arr[0, :, :, 2].T.copy())
        yl.append(arr[:, 0, :, 1].T.copy())   # [3, H]
        yh.append(arr[:, 0, :, 3].T.copy())
        wa.append(xh[-1][:, 0] - xl[-1][:, 0])        # [3]
        ha.append(yh[-1][:, 0] - yl[-1][:, 0])
        acx.append((xl[-1] + xh[-1]) * 0.5)
    return xl, xh, yl, yh, acx, wa, ha


def _prep_inputs(pred0, pred1, pred2, anchors0, anchors1, anchors2,
                 boxes, labels):
    B = pred0.shape[0]
    xl, xh, yl, yh, acx, wa, ha = _host_geometry(anchors0, anchors1,
                                                 anchors2)
    area9 = np.array([wa[s] * ha[s] for s in range(3)], np.float32)  # [3,3]

    # [3, 224] concat over scales then tile -> [120, 224]
    def cat3(v):
        return np.concatenate([v[0], v[1], v[2]], axis=1)  # [3, 224]

    # rows ordered a-major: row = a * NBOX + j
    xl3 = np.repeat(cat3(xl), NBOX, axis=0).astype(np.float32)
    xh3 = np.repeat(cat3(xh), NBOX, axis=0).astype(np.float32)
    yl3 = np.repeat(cat3(yl), NBOX, axis=0).astype(np.float32)
    yh3 = np.repeat(cat3(yh), NBOX, axis=0).astype(np.float32)
    msk3 = np.repeat(np.eye(3, dtype=np.float16), NBOX, axis=0)  # [120, 3]

    boxes = np.asarray(boxes, np.float32)
    labels = np.asarray(labels)
    bx1, by1, bx2, by2 = (boxes[..., 0], boxes[..., 1], boxes[..., 2],
                          boxes[..., 3])
    bw = bx2 - bx1
    bh = by2 - by1
    barea = bw * bh + 1e-9
    bcx = bx1 + 0.5 * bw
    bcy = by1 + 0.5 * bh
    lnwb = np.log(bw)
    lnhb = np.log(bh)

    boxc = np.zeros((B, 120, 4), np.float32)
    sc3 = np.zeros((B, 120, 3), np.float32)
    bbrow = np.zeros((B, 1, 200), np.float32)
    boxt = np.zeros((B, NBOX, 8), np.float32)
    for bi in range(B):
        for a in range(3):
            pr = a * NBOX + np.arange(NBOX)
            boxc[bi, pr, 0] = bx1[bi]
            boxc[bi, pr, 1] = bx2[bi]
            boxc[bi, pr, 2] = by1[bi]
            boxc[bi, pr, 3] = by2[bi]
            for s in range(3):
                sc3[bi, pr, s] = QSC / (area9[s, a] + barea[bi])
        bbrow[bi, 0, 0:40] = bx1[bi]
        bbrow[bi, 0, 40:80] = bx2[bi]
        bbrow[bi, 0, 80:120] = by1[bi]
        bbrow[bi, 0, 120:160] = by2[bi]
        bbrow[bi, 0, 160:200] = barea[bi]
        boxt[bi, :, 0] = bcx[bi]
        boxt[bi, :, 1] = bcy[bi]
        boxt[bi, :, 2] = lnwb[bi]
        boxt[bi, :, 3] = lnhb[bi]
        boxt[bi, :, 4] = labels[bi].astype(np.float32)

    # PREDI [B, 128*672, NF]: row p*672+col
    # fields: 0-3 deltas, 4-6 cls, 7 xl, 8 xh, 9 yl, 10 yh, 11 acx,
    #         12 rwa, 13 rha, 14 lnwa, 15 lnha, 16 A
    predi = np.zeros((B, P, NANCH, NF), np.float32)
    preds = [np.asarray(pred0, np.float32), np.asarray(pred1, np.float32),
             np.asarray(pred2, np.float32)]
    for s, (W, Hs, co, cw, yo, ywd) in enumerate(SC):
        pr = preds[s].reshape(B, 3, 8, Hs, W)
        blk = np.transpose(pr, (0, 3, 1, 4, 2))  # [B, y, a, x, ch]
        # deltas 0-3 -> fields 0-3; cls 5-7 -> fields 4-6
        predi[:, 0:Hs, co:co + cw, 0:4] = \
            blk[..., 0:4].reshape(B, Hs, 3 * W, 4)
        predi[:, 0:Hs, co:co + cw, 4:7] = \
            blk[..., 5:8].reshape(B, Hs, 3 * W, 3)
        for a in range(3):
            c0, c1 = co + a * W, co + (a + 1) * W
            predi[:, :, c0:c1, 7] = xl[s][a][None, None, :]
            predi[:, :, c0:c1, 8] = xh[s][a][None, None, :]
            predi[:, 0:Hs, c0:c1, 9] = yl[s][a][None, :, None]
            predi[:, 0:Hs, c0:c1, 10] = yh[s][a][None, :, None]
            predi[:, :, c0:c1, 11] = acx[s][a][None, None, :]
            predi[:, :, c0:c1, 12] = 1.0 / wa[s][a]
            predi[:, :, c0:c1, 13] = 1.0 / ha[s][a]
            predi[:, :, c0:c1, 14] = np.log(wa[s][a])
            predi[:, :, c0:c1, 15] = np.log(ha[s][a])
            predi[:, :, c0:c1, 16] = area9[s, a]
    predi = predi.reshape(B, P * NANCH, NF)

    return dict(xl3=xl3, xh3=xh3, yl3=yl3, yh3=yh3, msk3=msk3,
                boxc=boxc, sc3=sc3, bbrow=bbrow, boxt=boxt, predi=predi)


def kernel(pred0, pred1, pred2, anchors0, anchors1, anchors2, boxes,
           labels, _want_results=False, _trace=False):
    nc = _get_compiled()
    hp = _prep_inputs(pred0, pred1, pred2, anchors0, anchors1, anchors2,
                      boxes, labels)
    in_maps = []
    for c in range(NCORES):
        sl = slice(c * SPC, (c + 1) * SPC)
        in_maps.append({
            "pred0": np.ascontiguousarray(pred0[sl], np.float32),
            "pred1": np.ascontiguousarray(pred1[sl], np.float32),
            "pred2": np.ascontiguousarray(pred2[sl], np.float32),
            "predi0": np.ascontiguousarray(hp["predi"][c * SPC]),
            "predi1": np.ascontiguousarray(hp["predi"][c * SPC + 1]),
            "boxc": np.ascontiguousarray(hp["boxc"][sl]),
            "sc3": np.ascontiguousarray(hp["sc3"][sl]),
            "bbrow": np.ascontiguousarray(hp["bbrow"][sl]),
            "boxt0": np.ascontiguousarray(hp["boxt"][c * SPC]),
            "boxt1": np.ascontiguousarray(hp["boxt"][c * SPC + 1]),
            "xl3": hp["xl3"], "xh3": hp["xh3"],
            "yl3": hp["yl3"], "yh3": hp["yh3"], "msk3": hp["msk3"],
        })
    res = bass_utils.run_bass_kernel_spmd(
        nc, in_maps, core_ids=list(range(NCORES)), trace=_trace)
    parts = np.stack([res.results[c]["out"][:, 0] for c in range(NCORES)])
    tot = parts.sum(axis=0, dtype=np.float64)
    tot_obj = tot_cls = tot_loc = tot_pos = tot_neg = 0.0
    for b in range(SPC):
        pb = b * 8
        tot_obj += tot[pb + 0] + tot[pb + 6] + tot[pb + 7]
        tot_cls += tot[pb + 1]
        tot_loc += tot[pb + 2]
        tot_pos += tot[pb + 3] + tot[pb + 4] + tot[pb + 5]
        tot_neg += tot[16 + b]
    norm = np.float32(max(tot_pos, 1.0))
    lo = np.float32(tot_obj / norm)
    lc = np.float32(tot_cls / norm)
    ll = np.float32(tot_loc / norm)
    ltot = np.float32(lo + lc + np.float32(2.0) * ll)
    out = (lo, lc, ll, ltot, np.float32(tot_pos), np.float32(tot_neg))
    out = tuple(np.asarray(v, np.float32) for v in out)
    if _want_results:
        return out, res
    return out


# revision 18
# speedup vs baseline: 1.2090x; 1.0483x over previous
"""Trainium2 Bass kernel v2 for the 3-scale anchor DetectionLoss.

Data-parallel over batch: 16 samples -> 8 cores x 2 samples. Host sums
the per-core partial accumulators and applies the global normalizer.

Key structure (per sample):
- Anchor layout [128p, 672]: partition = grid row y; cols = s0 (3 sizes x
  128 x), s1 (3 x 64, rows 0..63), s2 (3 x 32, rows 0..31).
- IOU surrogate q = 64 * inter / (A + B): monotone in IOU, so argmax and
  the pos/neg thresholds (iou>=.5 <=> q>=64/3; iou<.3 <=> q<192/13)
  transfer. inter is separable: inter = fy(y) * fx(x), so the per-box
  pair stage is ONE rank-3 outer-product matmul per scale on the PE
  (lhsT = fy rows, rhs = block-diag fx pre-scaled by 64/(A+B)).
- ACT evacuates PSUM->fp16; DVE keeps a running max (BEST). pos/neg come
  from BEST; invalid (ragged) rows get BEST init 16.0, between the two
  scaled thresholds, so they are neither pos nor neg.
- Hard-negative mining: per-scale binary search for the k-th largest
  masked objectness loss; exact top-k sum via S(>thr) + (k-cnt)*thr.
- cls/loc losses only touch positives: per-partition top-16 positive
  columns are extracted with max8/match_replace/max_index, their data
  gathered via indirect DMA from host-interleaved DRAM tables, the
  matched box found by recomputing the 40-box q-strip per slot, and the
  small [128,16] tiles carry the SmoothL1 + CE math.
"""

import numpy as np
from contextlib import ExitStack

import concourse.bass as bass
import concourse.tile as tile
from concourse import bacc, mybir
from concourse import bass_utils
from concourse import bass_isa

F32 = mybir.dt.float32
F16 = mybir.dt.float16
U8 = mybir.dt.uint8
U16 = mybir.dt.uint16
U32 = mybir.dt.uint32
I32 = mybir.dt.int32
Alu = mybir.AluOpType
Act = mybir.ActivationFunctionType
Ax = mybir.AxisListType

NCORES = 8
SPC = 2
NBOX = 40
P = 128
NSLOT = 16
NITER = 9
QSC = 64.0                  # q scale to keep 1/(A+B) in fp16 normal range
POS_THR = QSC / 3.0         # q >= this  <=> iou >= 0.5
NEG_THR = QSC * 0.3 / 1.3   # q <  this  <=> iou < 0.3
GARB = 16.0                 # between NEG_THR (14.77) and POS_THR (21.33)

# scale: (W, H, fxd col off, fxd width, y-block off, y-width)
SC = [(128, 128, 0, 384, 0, 128), (64, 64, 384, 192, 128, 64),
      (32, 32, 576, 96, 192, 32)]
NANCH = 672                 # anchor cols per sample tile
NF = 20                     # fields per PREDI row
# PREDI fields: 0-3 deltas, 4-6 cls logits, 7 xl, 8 xh, 9 yl, 10 yh,
# 11 acx, 12 rwa, 13 rha, 14 lnwa, 15 lnha, 16 A


def bc_ins(ap, dim, n):
    """Insert a stride-0 dim of size n at position dim."""
    layout = [list(d) for d in ap.ap]
    layout.insert(dim, [0, n])
    return bass.AP(ap.tensor, ap.offset, layout)


def _build_body(tc, aps):
    nc = tc.nc
    dve = nc.vector
    act = nc.scalar
    gp = nc.gpsimd

    with ExitStack() as ctx:
        pc = ctx.enter_context(tc.tile_pool(name="const", bufs=1))
        pp = ctx.enter_context(tc.tile_pool(name="prep", bufs=2))
        pq = ctx.enter_context(tc.tile_pool(name="qpair", bufs=1))
        pt = ctx.enter_context(tc.tile_pool(name="ptrans", bufs=1))
        pqs = ctx.enter_context(tc.tile_pool(name="qpsum", bufs=1,
                                             space="PSUM"))
        pd = ctx.enter_context(tc.tile_pool(name="dense", bufs=2))
        psl = ctx.enter_context(tc.tile_pool(name="slots", bufs=2))
        pmi = ctx.enter_context(tc.tile_pool(name="mine", bufs=2))
        pfin = ctx.enter_context(tc.tile_pool(name="fin", bufs=1))
        pps = ctx.enter_context(tc.tile_pool(name="smallps", bufs=1,
                                             space="PSUM"))

        # ---------- per-kernel constants ----------
        XL3 = pc.tile([120, 224], F32, tag="xl3", name="xl3")
        XH3 = pc.tile([120, 224], F32, tag="xh3", name="xh3")
        YL3 = pc.tile([120, 224], F32, tag="yl3", name="yl3")
        YH3 = pc.tile([120, 224], F32, tag="yh3", name="yh3")
        MSK3 = pc.tile([120, 3], F16, tag="msk3", name="msk3")
        for t, k in ((XL3, "xl3"), (XH3, "xh3"), (YL3, "yl3"),
                     (YH3, "yh3"), (MSK3, "msk3")):
            nc.sync.dma_start(t[:], aps[k])

        ONESC = pc.tile([P, 1], F32, tag="onesc", name="onesc")
        dve.memset(ONESC[:], 1.0)

        KEYW = pc.tile([P, NANCH], F16, tag="keyw", name="keyw")
        JIF = pc.tile([P, NSLOT * NBOX], F32, tag="jif", name="jif")
        POF = pc.tile([P, 1], F32, tag="pof", name="pof")
        with tc.tile_pool(name="initscr", bufs=1) as pin:
            kwi = pin.tile([P, NANCH], I32, tag="kwi", name="kwi")
            gp.iota(kwi[:], [[1, NANCH]], base=0, channel_multiplier=0)
            kwf = pin.tile([P, NANCH], F32, tag="kwf", name="kwf")
            dve.tensor_copy(kwf[:], kwi[:])
            dve.tensor_scalar(KEYW[:], kwf[:], -1.0 / 2048.0, 1.0,
                              Alu.mult, Alu.add)
            ji = pin.tile([P, NSLOT * NBOX], I32, tag="ji", name="ji")
            gp.iota(ji[:], [[0, NSLOT], [1, NBOX]], base=0,
                    channel_multiplier=0)
            dve.tensor_copy(JIF[:], ji[:])
            pofi = pin.tile([P, 1], I32, tag="pofi", name="pofi")
            gp.iota(pofi[:], [[1, 1]], base=0, channel_multiplier=NANCH)
            dve.tensor_copy(POF[:], pofi[:])      # p * 672

        PART = pfin.tile([P, 18], F32, tag="part", name="part")
        dve.memset(PART[:], 0.0)

        S = [dict(), dict()]

        # ================= stages =================
        def prep(b):
            st = S[b]
            BOXC = pp.tile([120, 4], F32, tag="boxc", name=f"boxc{b}")
            nc.sync.dma_start(BOXC[:], aps["boxc"][b])
            SC3 = pp.tile([120, 3], F32, tag="sc3", name=f"sc3{b}")
            nc.sync.dma_start(SC3[:], aps["sc3"][b])
            t1 = pp.tile([120, 224], F32, tag="t1", name=f"t1{b}")
            t2 = pp.tile([120, 224], F32, tag="t2", name=f"t2{b}")
            FYJ = pp.tile([120, 224], F16, tag="fyj", name=f"fyj{b}")
            dve.tensor_scalar(t1[:], YH3[:], BOXC[:, 3:4], None, Alu.min)
            dve.tensor_scalar(t2[:], YL3[:], BOXC[:, 2:3], None, Alu.max)
            dve.tensor_tensor(t1[:], t1[:], t2[:], Alu.subtract)
            act.activation(FYJ[:], t1[:], Act.Relu)
            fx1 = pp.tile([120, 224], F32, tag="fx1", name=f"fx1{b}")
            fx2 = pp.tile([120, 224], F32, tag="fx2", name=f"fx2{b}")
            FXS = pp.tile([120, 224], F16, tag="fxs", name=f"fxs{b}")
            dve.tensor_scalar(fx1[:], XH3[:], BOXC[:, 1:2], None, Alu.min)
            dve.tensor_scalar(fx2[:], XL3[:], BOXC[:, 0:1], None, Alu.max)
            dve.tensor_tensor(fx1[:], fx1[:], fx2[:], Alu.subtract)
            xo = 0
            for s, (W, H, co, cw, yo, yw) in enumerate(SC):
                act.activation(FXS[:, xo:xo + W], fx1[:, xo:xo + W],
                               Act.Relu, scale=SC3[:, s:s + 1])
                xo += W
            FXD = pp.tile([120, NANCH], F16, tag="fxd", name=f"fxd{b}")
            xo = 0
            for s, (W, H, co, cw, yo, yw) in enumerate(SC):
                srcv = bc_ins(FXS[:, xo:xo + W], 1, 3)
                msk = MSK3[:].to_broadcast([120, 3, W])
                dve.tensor_tensor(
                    FXD[:, co:co + cw].rearrange("p (a x) -> p a x", a=3),
                    srcv, msk, Alu.mult)
                xo += W
            POBJ = pd.tile([P, NANCH], F32, tag="pobj", name=f"pobj{b}")
            dve.memset(POBJ[64:128, 384:576], 0.0)
            dve.memset(POBJ[32:64, 576:672], 0.0)
            dve.memset(POBJ[64:128, 576:672], 0.0)
            preds = [aps["pred0"], aps["pred1"], aps["pred2"]]
            for s, (W, H, co, cw, yo, yw) in enumerate(SC):
                for a in range(3):
                    nc.sync.dma_start(
                        POBJ[0:H, co + a * W: co + (a + 1) * W],
                        preds[s][b, a * 8 + 4])
            BESTe = pd.tile([P, NANCH], F16, tag="beste", name=f"beste{b}")
            BESTo = pd.tile([P, NANCH], F16, tag="besto", name=f"besto{b}")
            for t in (BESTe, BESTo):
                dve.memset(t[:, 0:384], 0.0)
                dve.memset(t[0:64, 384:576], 0.0)
                dve.memset(t[0:32, 576:672], 0.0)
                dve.memset(t[64:128, 384:576], GARB)
                dve.memset(t[32:64, 576:672], GARB)
                dve.memset(t[64:128, 576:672], GARB)
            st.update(FYJ=FYJ, FXD=FXD, POBJ=POBJ, BESTe=BESTe, BESTo=BESTo)

        CB = 10

        def pair_chunk(b, ch):
            st = S[b]
            FYJ, FXD = st["FYJ"], st["FXD"]
            BESTe, BESTo = st["BESTe"], st["BESTo"]
            FYT = pt.tile([3, CB * 224], F16, tag=f"fyt{b}{ch % 2}",
                          name=f"fyt{b}_{ch}")
            FXT = pt.tile([3, CB * NANCH], F16, tag=f"fxt{b}{ch % 2}",
                          name=f"fxt{b}_{ch}")
            for a in range(3):
                rows = slice(40 * a + CB * ch, 40 * a + CB * (ch + 1))
                nc.sync.dma_start(FYT[a:a + 1, :], FYJ[rows, :])
                nc.sync.dma_start(FXT[a:a + 1, :], FXD[rows, :])
            for jj in range(CB):
                j = ch * CB + jj
                yo = jj * 224
                xo = jj * NANCH
                psA = pqs.tile([P, 384], F32, tag=f"psA{b}{j % 2}",
                               name=f"psA{b}_{j}")
                psB = pqs.tile([64, 288], F32, tag=f"psB{j % 2}",
                               name=f"psB{b}_{j}")
                nc.tensor.matmul(psA[:], FYT[0:3, yo:yo + 128],
                                 FXT[0:3, xo:xo + 384],
                                 start=True, stop=True)
                nc.tensor.matmul(psB[0:64, 0:192],
                                 FYT[0:3, yo + 128:yo + 192],
                                 FXT[0:3, xo + 384:xo + 576],
                                 start=True, stop=True)
                nc.tensor.matmul(psB[0:32, 192:288],
                                 FYT[0:3, yo + 192:yo + 224],
                                 FXT[0:3, xo + 576:xo + 672],
                                 start=True, stop=True)
                QA = pq.tile([P, 384], F16, tag=f"qa{b}{j % 2}",
                             name=f"qa{b}_{j}")
                QB = pq.tile([64, 288], F16, tag=f"qb{b}{j % 2}",
                             name=f"qb{b}_{j}")
                act.activation(QA[:], psA[:], Act.Copy)
                act.activation(QB[0:64, 0:192], psB[0:64, 0:192], Act.Copy)
                act.activation(QB[0:32, 192:288], psB[0:32, 192:288],
                               Act.Copy)
                acc = BESTe if j % 2 == 0 else BESTo
                dve.tensor_tensor(acc[:, 0:384], acc[:, 0:384], QA[:],
                                  Alu.max)
                dve.tensor_tensor(acc[0:64, 384:576], acc[0:64, 384:576],
                                  QB[0:64, 0:192], Alu.max)
                dve.tensor_tensor(acc[0:32, 576:672], acc[0:32, 576:672],
                                  QB[0:32, 192:288], Alu.max)

        def dense(b):
            st = S[b]
            pb = b * 8
            POBJ = st["POBJ"]
            BEST = pd.tile([P, NANCH], F16, tag="best", name=f"best{b}")
            dve.tensor_tensor(BEST[:], st["BESTe"][:], st["BESTo"][:],
                              Alu.max)
            POS = pd.tile([P, NANCH], F16, tag="pos", name=f"pos{b}")
            NEG = pd.tile([P, NANCH], F16, tag="neg", name=f"neg{b}")
            dve.tensor_scalar(POS[:], BEST[:], POS_THR, None, Alu.is_ge)
            dve.tensor_scalar(NEG[:], BEST[:], NEG_THR, None, Alu.is_lt)
            AX = pd.tile([P, NANCH], F32, tag="ax", name=f"ax{b}")
            SP = pd.tile([P, NANCH], F32, tag="sp", name=f"sp{b}")
            act.activation(AX[:], POBJ[:], Act.Abs)
            act.activation(AX[:], AX[:], Act.Exp, scale=-1.0)
            act.activation(AX[:], AX[:], Act.Ln, bias=1.0)
            act.activation(SP[:], POBJ[:], Act.Relu)
            dve.tensor_tensor(SP[:], SP[:], AX[:], Alu.add)
            dve.tensor_tensor(POBJ[:], SP[:], POBJ[:], Alu.subtract)
            scr = pd.tile([P, NANCH], F32, tag="scr", name=f"scr{b}")
            dve.tensor_tensor(scr[:], POBJ[:], POS[:], Alu.mult)
            dve.tensor_scalar(AX[:], scr[:], 0.0, 0.0, Alu.add, Alu.add,
                              accum_out=PART[:, pb:pb + 1])
            NEGL = pd.tile([P, NANCH], F16, tag="negl", name=f"negl{b}")
            dve.tensor_tensor(NEGL[:], NEG[:], SP[:], Alu.mult)
            mc16a = pd.tile([P, NANCH], F16, tag="mc16a", name=f"mc16a{b}")
            CNT = pmi.tile([P, 8], F32, tag="cnt", name=f"cnt{b}")
            for s, (W, H, co, cw, yo, yw) in enumerate(SC):
                blk = slice(co, co + cw)
                dve.tensor_scalar(mc16a[:, blk], POS[:, blk], 0.0, 0.0,
                                  Alu.add, Alu.add,
                                  accum_out=CNT[:, s:s + 1])
                dve.tensor_scalar(mc16a[:, blk], NEG[:, blk], 0.0, 0.0,
                                  Alu.add, Alu.add,
                                  accum_out=CNT[:, 4 + s:5 + s])
            NPOS3 = pmi.tile([P, 3], F32, tag="npos3", name=f"npos3{b}")
            NNEG3 = pmi.tile([P, 3], F32, tag="nneg3", name=f"nneg3{b}")
            gp.partition_all_reduce(NPOS3[:], CNT[:, 0:3], P,
                                    bass_isa.ReduceOp.add)
            gp.partition_all_reduce(NNEG3[:], CNT[:, 4:7], P,
                                    bass_isa.ReduceOp.add)
            dve.tensor_copy(PART[0:1, pb + 3:pb + 6], NPOS3[0:1, :])
            st.update(BEST=BEST, POS=POS, NEG=NEG, SP=SP, NEGL=NEGL,
                      NPOS3=NPOS3, NNEG3=NNEG3)

        def mine_init(b):
            st = S[b]
            NEGL = st["NEGL"]
            K3 = pmi.tile([P, 3], F32, tag="k3", name=f"k3{b}")
            dve.tensor_scalar(K3[:], st["NPOS3"][:], 1.0, 3.0, Alu.max,
                              Alu.mult)
            dve.tensor_tensor(K3[:], K3[:], st["NNEG3"][:], Alu.min)
            HI3 = pmi.tile([P, 3], F32, tag="hi3", name=f"hi3{b}")
            LO3 = pmi.tile([P, 3], F32, tag="lo3", name=f"lo3{b}")
            RM3 = pmi.tile([P, 3], F32, tag="rm3", name=f"rm3{b}")
            for s, (W, H, co, cw, yo, yw) in enumerate(SC):
                dve.tensor_reduce(RM3[:, s:s + 1], NEGL[:, co:co + cw],
                                  Ax.X, Alu.max)
            gp.partition_all_reduce(HI3[:], RM3[:], P,
                                    bass_isa.ReduceOp.max)
            dve.memset(LO3[:], 0.0)
            MID3 = pmi.tile([P, 3], F32, tag="mid3", name=f"mid3{b}")
            CP3 = pmi.tile([P, 3], F32, tag="cp3", name=f"cp3{b}")
            CT3 = pmi.tile([P, 3], F32, tag="ct3", name=f"ct3{b}")
            GTK = pmi.tile([P, 3], U8, tag="gtk", name=f"gtk{b}")
            LEK = pmi.tile([P, 3], U8, tag="lek", name=f"lek{b}")
            mc16 = pd.tile([P, NANCH], F16, tag="mc16", name=f"mc16{b}")
            st.update(K3=K3, HI3=HI3, LO3=LO3, MID3=MID3, CP3=CP3,
                      CT3=CT3, GTK=GTK, LEK=LEK, mc16=mc16)

        def mine_iter(b):
            st = S[b]
            NEGL, mc16 = st["NEGL"], st["mc16"]
            K3, HI3, LO3, MID3 = st["K3"], st["HI3"], st["LO3"], st["MID3"]
            CP3, CT3, GTK, LEK = st["CP3"], st["CT3"], st["GTK"], st["LEK"]
            dve.tensor_tensor(MID3[:], LO3[:], HI3[:], Alu.add)
            dve.tensor_scalar(MID3[:], MID3[:], 0.5, None, Alu.mult)
            for s, (W, H, co, cw, yo, yw) in enumerate(SC):
                blk = slice(co, co + cw)
                dve.tensor_scalar(mc16[:, blk], NEGL[:, blk],
                                  MID3[:, s:s + 1], 0.0, Alu.is_gt,
                                  Alu.add, accum_out=CP3[:, s:s + 1])
            gp.partition_all_reduce(CT3[:], CP3[:], P,
                                    bass_isa.ReduceOp.add)
            dve.tensor_tensor(GTK[:], CT3[:], K3[:], Alu.is_gt)
            dve.tensor_tensor(LEK[:], CT3[:], K3[:], Alu.is_le)
            dve.copy_predicated(LO3[:], GTK[:], MID3[:])
            dve.copy_predicated(HI3[:], LEK[:], MID3[:])

        def mine_fin(b):
            st = S[b]
            pb = b * 8
            NEGL, mc16 = st["NEGL"], st["mc16"]
            K3, HI3, CP3, CT3 = st["K3"], st["HI3"], st["CP3"], st["CT3"]
            SG3 = pmi.tile([P, 3], F32, tag="sg3", name=f"sg3{b}")
            for s, (W, H, co, cw, yo, yw) in enumerate(SC):
                blk = slice(co, co + cw)
                dve.tensor_scalar(mc16[:, blk], NEGL[:, blk],
                                  HI3[:, s:s + 1], 0.0, Alu.is_gt,
                                  Alu.add, accum_out=CP3[:, s:s + 1])
                dve.tensor_tensor(mc16[:, blk], NEGL[:, blk], mc16[:, blk],
                                  Alu.mult)
                dve.tensor_scalar(mc16[:, blk], mc16[:, blk], 0.0, 0.0,
                                  Alu.add, Alu.add,
                                  accum_out=SG3[:, s:s + 1])
            gp.partition_all_reduce(CT3[:], CP3[:], P,
                                    bass_isa.ReduceOp.add)
            dve.tensor_copy(PART[:, pb + 6:pb + 7], SG3[:, 0:1])
            dve.tensor_tensor(PART[:, pb + 6:pb + 7],
                              PART[:, pb + 6:pb + 7], SG3[:, 1:2], Alu.add)
            dve.tensor_tensor(PART[:, pb + 6:pb + 7],
                              PART[:, pb + 6:pb + 7], SG3[:, 2:3], Alu.add)
            TK = pmi.tile([P, 3], F32, tag="tk", name=f"tk{b}")
            dve.tensor_tensor(TK[:], K3[:], CT3[:], Alu.subtract)
            dve.tensor_tensor(TK[:], TK[:], HI3[:], Alu.mult)
            dve.tensor_copy(PART[0:1, pb + 7:pb + 8], TK[0:1, 0:1])
            dve.tensor_tensor(PART[0:1, pb + 7:pb + 8],
                              PART[0:1, pb + 7:pb + 8], TK[0:1, 1:2],
                              Alu.add)
            dve.tensor_tensor(PART[0:1, pb + 7:pb + 8],
                              PART[0:1, pb + 7:pb + 8], TK[0:1, 2:3],
                              Alu.add)
            KS = pmi.tile([P, 1], F32, tag="ks", name=f"ks{b}")
            dve.tensor_copy(KS[:], K3[:, 0:1])
            dve.tensor_tensor(KS[:], KS[:], K3[:, 1:2], Alu.add)
            dve.tensor_tensor(KS[:], KS[:], K3[:, 2:3], Alu.add)
            dve.tensor_copy(PART[0:1, 16 + b:17 + b], KS[0:1, :])

        def slots_a(b):
            st = S[b]
            POS = st["POS"]
            KEY = psl.tile([P, NANCH], F16, tag="key", name=f"key{b}")
            dve.tensor_tensor(KEY[:], POS[:], KEYW[:], Alu.mult)
            K8a = psl.tile([P, 8], F16, tag="k8a", name=f"k8a{b}")
            K8b = psl.tile([P, 8], F16, tag="k8b", name=f"k8b{b}")
            IX8a = psl.tile([P, 8], U16, tag="ix8a", name=f"ix8a{b}")
            IX8b = psl.tile([P, 8], U16, tag="ix8b", name=f"ix8b{b}")
            KEY2 = psl.tile([P, NANCH], F16, tag="key2", name=f"key2{b}")
            dve.max(K8a[:], KEY[:])
            dve.max_index(IX8a[:], K8a[:], KEY[:])
            dve.match_replace(KEY2[:], K8a[:], KEY[:], -1.0)
            dve.max(K8b[:], KEY2[:])
            dve.max_index(IX8b[:], K8b[:], KEY2[:])
            VAL = psl.tile([P, NSLOT], F16, tag="val", name=f"val{b}")
            dve.tensor_scalar(VAL[:, 0:8], K8a[:], 0.0, None, Alu.is_gt)
            dve.tensor_scalar(VAL[:, 8:16], K8b[:], 0.0, None, Alu.is_gt)
            COLU = psl.tile([P, NSLOT], U32, tag="colu", name=f"colu{b}")
            dve.tensor_copy(COLU[:, 0:8], IX8a[:])
            dve.tensor_copy(COLU[:, 8:16], IX8b[:])
            COLF = psl.tile([P, NSLOT], F32, tag="colf", name=f"colf{b}")
            dve.tensor_copy(COLF[:], COLU[:])
            OFFP = psl.tile([P, NSLOT], F32, tag="offp", name=f"offp{b}")
            dve.tensor_scalar(OFFP[:], COLF[:], POF[:, 0:1], 0.0, Alu.add,
                              Alu.add)
            OFFPU = psl.tile([P, NSLOT], U32, tag="offpu", name=f"offpu{b}")
            dve.tensor_copy(OFFPU[:], OFFP[:])
            GSA = psl.tile([P, NSLOT * NF], F32, tag="gsa", name=f"gsa{b}")
            for s in range(NSLOT):
                ofs = psl.tile([P, 1], U32, tag=f"ofs{s}",
                               name=f"ofs{b}_{s}")
                dve.tensor_copy(ofs[:], OFFPU[:, s:s + 1])
                gp.indirect_dma_start(
                    out=GSA[:, s * NF:(s + 1) * NF], out_offset=None,
                    in_=aps[f"predi{b}"][:],
                    in_offset=bass.IndirectOffsetOnAxis(ap=ofs[:], axis=0))
            BB = psl.tile([P, 200], F32, tag="bb", name=f"bb{b}")
            nc.sync.dma_start(BB[:], bc_ins(aps["bbrow"][b][0], 0, P))
            st.update(VAL=VAL, GSA=GSA, BB=BB)

        def slots_b(b):
            st = S[b]
            GSA, BB = st["GSA"], st["BB"]
            GT = psl.tile([P, NF * NSLOT], F32, tag="gt", name=f"gt{b}")
            dve.tensor_copy(GT[:].rearrange("p (f s) -> p s f", s=NSLOT),
                            GSA[:].rearrange("p (s f) -> p s f", f=NF))

            def fld(fi):
                return GT[:, fi * NSLOT:(fi + 1) * NSLOT]

            XLs, XHs, YLs, YHs = fld(7), fld(8), fld(9), fld(10)
            AAs = fld(16)
            SJ = NSLOT * NBOX

            def strip_ov(name, lo_ap, hi_ap, blo, bhi):
                m1 = psl.tile([P, SJ], F32, tag="sv1", name=f"{name}1{b}")
                m2 = psl.tile([P, SJ], F32, tag="sv2", name=f"{name}2{b}")
                v3 = m1[:].rearrange("p (s j) -> p s j", j=NBOX)
                v4 = m2[:].rearrange("p (s j) -> p s j", j=NBOX)
                dve.tensor_tensor(v3, hi_ap, bhi, Alu.min)
                dve.tensor_tensor(v4, lo_ap, blo, Alu.max)
                dve.tensor_tensor(m1[:], m1[:], m2[:], Alu.subtract)
                r = psl.tile([P, SJ], F32, tag=f"{name}r",
                             name=f"{name}r{b}")
                act.activation(r[:], m1[:], Act.Relu)
                return r

            xl_b = XLs.to_broadcast([P, NSLOT, NBOX])
            xh_b = XHs.to_broadcast([P, NSLOT, NBOX])
            yl_b = YLs.to_broadcast([P, NSLOT, NBOX])
            yh_b = YHs.to_broadcast([P, NSLOT, NBOX])
            bx1_b = bc_ins(BB[:, 0:40], 1, NSLOT)
            bx2_b = bc_ins(BB[:, 40:80], 1, NSLOT)
            by1_b = bc_ins(BB[:, 80:120], 1, NSLOT)
            by2_b = bc_ins(BB[:, 120:160], 1, NSLOT)
            FXP = strip_ov("fx", xl_b, xh_b, bx1_b, bx2_b)
            FYP = strip_ov("fy", yl_b, yh_b, by1_b, by2_b)
            ABJ = psl.tile([P, SJ], F32, tag="abj", name=f"abj{b}")
            dve.tensor_tensor(ABJ[:].rearrange("p (s j) -> p s j", j=NBOX),
                              AAs.to_broadcast([P, NSLOT, NBOX]),
                              bc_ins(BB[:, 160:200], 1, NSLOT), Alu.add)
            SRJ = psl.tile([P, SJ], F32, tag="srj", name=f"srj{b}")
            dve.reciprocal_approx_fast(SRJ[:], ABJ[:])
            QST = FXP
            dve.tensor_tensor(QST[:], QST[:], FYP[:], Alu.mult)
            dve.tensor_tensor(QST[:], QST[:], SRJ[:], Alu.mult)
            BQ = psl.tile([P, NSLOT], F32, tag="bq", name=f"bq{b}")
            dve.tensor_reduce(BQ[:], QST[:].rearrange(
                "p (s j) -> p s j", j=NBOX), Ax.X, Alu.max)
            MSKJ = psl.tile([P, SJ], U8, tag="mskj", name=f"mskj{b}")
            dve.tensor_tensor(MSKJ[:].rearrange("p (s j) -> p s j", j=NBOX),
                              QST[:].rearrange("p (s j) -> p s j", j=NBOX),
                              BQ[:].to_broadcast([P, NSLOT, NBOX]),
                              Alu.is_ge)
            JM = psl.tile([P, SJ], F32, tag="jm", name=f"jm{b}")
            dve.memset(JM[:], 99.0)
            dve.copy_predicated(JM[:], MSKJ[:], JIF[:])
            JF = psl.tile([P, NSLOT], F32, tag="jf", name=f"jf{b}")
            dve.tensor_reduce(JF[:], JM[:].rearrange(
                "p (s j) -> p s j", j=NBOX), Ax.X, Alu.min)
            JU = psl.tile([P, NSLOT], U32, tag="ju", name=f"ju{b}")
            dve.tensor_copy(JU[:], JF[:])
            BVA = psl.tile([P, NSLOT * 8], F32, tag="bva", name=f"bva{b}")
            for s in range(NSLOT):
                ofj = psl.tile([P, 1], U32, tag=f"ofj{s}",
                               name=f"ofj{b}_{s}")
                dve.tensor_copy(ofj[:], JU[:, s:s + 1])
                gp.indirect_dma_start(
                    out=BVA[:, s * 8:(s + 1) * 8], out_offset=None,
                    in_=aps[f"boxt{b}"][:],
                    in_offset=bass.IndirectOffsetOnAxis(ap=ofj[:], axis=0))
            BVT = psl.tile([P, 8 * NSLOT], F32, tag="bvt", name=f"bvt{b}")
            dve.tensor_copy(BVT[:].rearrange("p (f s) -> p s f", s=NSLOT),
                            BVA[:].rearrange("p (s f) -> p s f", f=8))
            st.update(GT=GT, BVT=BVT)

        def slots_c(b):
            st = S[b]
            pb = b * 8
            GT, BVT, VAL = st["GT"], st["BVT"], st["VAL"]

            def fld(fi):
                return GT[:, fi * NSLOT:(fi + 1) * NSLOT]

            def bfld(fi):
                return BVT[:, fi * NSLOT:(fi + 1) * NSLOT]

            def stt(name):
                return psl.tile([P, NSLOT], F32, tag=name,
                                name=f"{name}{b}")

            YLs, YHs = fld(9), fld(10)
            ACXs, RWAs, RHAs, LNWAs, LNHAs = (fld(11), fld(12), fld(13),
                                              fld(14), fld(15))
            BCXs, BCYs, LNWs, LNHs, LABs = (bfld(0), bfld(1), bfld(2),
                                            bfld(3), bfld(4))
            ACYs = stt("acys")
            dve.tensor_tensor(ACYs[:], YLs, YHs, Alu.add)
            dve.tensor_scalar(ACYs[:], ACYs[:], 0.5, None, Alu.mult)
            encs = []
            e0 = stt("e0")
            dve.tensor_tensor(e0[:], BCXs, ACXs, Alu.subtract)
            dve.tensor_tensor(e0[:], e0[:], RWAs, Alu.mult)
            encs.append(e0)
            e1 = stt("e1")
            dve.tensor_tensor(e1[:], BCYs, ACYs[:], Alu.subtract)
            dve.tensor_tensor(e1[:], e1[:], RHAs, Alu.mult)
            encs.append(e1)
            e2 = stt("e2")
            dve.tensor_tensor(e2[:], LNWs, LNWAs, Alu.subtract)
            encs.append(e2)
            e3 = stt("e3")
            dve.tensor_tensor(e3[:], LNHs, LNHAs, Alu.subtract)
            encs.append(e3)
            SL = stt("sl")
            first = True
            for c in range(4):
                d = stt(f"d{c}")
                dve.tensor_tensor(d[:], fld(c), encs[c][:], Alu.subtract)
                ad = stt(f"ad{c}")
                act.activation(ad[:], d[:], Act.Abs)
                mm = stt(f"mm{c}")
                dve.tensor_scalar(mm[:], ad[:], 1.0, None, Alu.min)
                q1 = stt(f"q1{c}")
                dve.tensor_tensor(q1[:], mm[:], mm[:], Alu.mult)
                dve.tensor_scalar(q1[:], q1[:], 0.5, None, Alu.mult)
                u1 = stt(f"u1{c}")
                dve.tensor_tensor(u1[:], ad[:], mm[:], Alu.subtract)
                dve.tensor_tensor(q1[:], q1[:], u1[:], Alu.add)
                if first:
                    dve.tensor_copy(SL[:], q1[:])
                    first = False
                else:
                    dve.tensor_tensor(SL[:], SL[:], q1[:], Alu.add)
            lscr = stt("lscr")
            dve.scalar_tensor_tensor(lscr[:], SL[:], 0.0, VAL[:], Alu.add,
                                     Alu.mult,
                                     accum_out=PART[:, pb + 2:pb + 3])
            c0, c1, c2 = fld(4), fld(5), fld(6)
            mx = stt("mx")
            dve.tensor_tensor(mx[:], c0, c1, Alu.max)
            dve.tensor_tensor(mx[:], mx[:], c2, Alu.max)
            ssum = stt("ssum")
            first = True
            for ci, cap in enumerate((c0, c1, c2)):
                dd = stt(f"dd{ci}")
                dve.tensor_tensor(dd[:], cap, mx[:], Alu.subtract)
                ee = stt(f"ee{ci}")
                act.activation(ee[:], dd[:], Act.Exp)
                if first:
                    dve.tensor_copy(ssum[:], ee[:])
                    first = False
                else:
                    dve.tensor_tensor(ssum[:], ssum[:], ee[:], Alu.add)
            lse = stt("lse")
            act.activation(lse[:], ssum[:], Act.Ln)
            dve.tensor_tensor(lse[:], lse[:], mx[:], Alu.add)
            pick = stt("pick")
            dve.tensor_copy(pick[:], c0)
            m1u = psl.tile([P, NSLOT], U8, tag="m1u", name=f"m1u{b}")
            m2u = psl.tile([P, NSLOT], U8, tag="m2u", name=f"m2u{b}")
            dve.tensor_scalar(m1u[:], LABs, 2.0, None, Alu.is_equal)
            dve.tensor_scalar(m2u[:], LABs, 3.0, None, Alu.is_equal)
            dve.copy_predicated(pick[:], m1u[:], c1)
            dve.copy_predicated(pick[:], m2u[:], c2)
            ce = stt("ce")
            dve.tensor_tensor(ce[:], lse[:], pick[:], Alu.subtract)
            cscr = stt("cscr")
            dve.scalar_tensor_tensor(cscr[:], ce[:], 0.0, VAL[:], Alu.add,
                                     Alu.mult,
                                     accum_out=PART[:, pb + 1:pb + 2])

        # ================= schedule =================
        for b in range(SPC):
            prep(b)
        for ch in range(NBOX // CB):
            for b in range(SPC):
                pair_chunk(b, ch)
        for b in range(SPC):
            dense(b)
        for b in range(SPC):
            mine_init(b)
            slots_a(b)
        for it in range(NITER):
            for b in range(SPC):
                mine_iter(b)
        for b in range(SPC):
            slots_b(b)
        for b in range(SPC):
            mine_fin(b)
            slots_c(b)

        # ---------- final partition reduction ----------
        fin = pps.tile([18, 1], F32, tag="fin", name="fin")
        nc.tensor.matmul(fin[:], PART[:], ONESC[:], start=True, stop=True)
        OUTT = pfin.tile([18, 1], F32, tag="outt", name="outt")
        act.activation(OUTT[:], fin[:], Act.Copy)
        nc.sync.dma_start(aps["out"], OUTT[:])


_CACHE = {}


def _get_compiled():
    if "nc" in _CACHE:
        return _CACHE["nc"]
    nc = bacc.Bacc("TRN2", target_bir_lowering=False, debug=False)
    aps = {
        "pred0": nc.dram_tensor("pred0", [SPC, 24, 128, 128], F32,
                                kind="ExternalInput").ap(),
        "pred1": nc.dram_tensor("pred1", [SPC, 24, 64, 64], F32,
                                kind="ExternalInput").ap(),
        "pred2": nc.dram_tensor("pred2", [SPC, 24, 32, 32], F32,
                                kind="ExternalInput").ap(),
        "predi0": nc.dram_tensor("predi0", [P * NANCH, NF], F32,
                                 kind="ExternalInput").ap(),
        "predi1": nc.dram_tensor("predi1", [P * NANCH, NF], F32,
                                 kind="ExternalInput").ap(),
        "boxc": nc.dram_tensor("boxc", [SPC, 120, 4], F32,
                               kind="ExternalInput").ap(),
        "sc3": nc.dram_tensor("sc3", [SPC, 120, 3], F32,
                              kind="ExternalInput").ap(),
        "bbrow": nc.dram_tensor("bbrow", [SPC, 1, 200], F32,
                                kind="ExternalInput").ap(),
        "boxt0": nc.dram_tensor("boxt0", [NBOX, 8], F32,
                                kind="ExternalInput").ap(),
        "boxt1": nc.dram_tensor("boxt1", [NBOX, 8], F32,
                                kind="ExternalInput").ap(),
        "xl3": nc.dram_tensor("xl3", [120, 224], F32,
                              kind="ExternalInput").ap(),
        "xh3": nc.dram_tensor("xh3", [120, 224], F32,
                              kind="ExternalInput").ap(),
        "yl3": nc.dram_tensor("yl3", [120, 224], F32,
                              kind="ExternalInput").ap(),
        "yh3": nc.dram_tensor("yh3", [120, 224], F32,
                              kind="ExternalInput").ap(),
        "msk3": nc.dram_tensor("msk3", [120, 3], F16,
                               kind="ExternalInput").ap(),
        "out": nc.dram_tensor("out", [18, 1], F32,
                              kind="ExternalOutput").ap(),
    }
    with tile.TileContext(nc) as tc:
        _build_body(tc, aps)
    nc.compile()
    _CACHE["nc"] = nc
    return nc


def _host_geometry(anchors0, anchors1, anchors2):
    """Extract per-axis marginals from the grid-structured anchors."""
    HW = [(128, 128), (64, 64), (32, 32)]
    ancs = [np.asarray(anchors0, np.float32),
            np.asarray(anchors1, np.float32),
            np.asarray(anchors2, np.float32)]
    xl, xh, yl, yh, acx, wa, ha = [], [], [], [], [], [], []
    for (H, W), anc in zip(HW, ancs):
        arr = anc.reshape(H, W, 3, 4)
        xl.append(arr[0, :, :, 0].T.copy())   # [3, W]
        xh.append(arr[0, :, :, 2].T.copy())
        yl.append(arr[:, 0, :, 1].T.copy())   # [3, H]
        yh.append(arr[:, 0, :, 3].T.copy())
        wa.append(xh[-1][:, 0] - xl[-1][:, 0])        # [3]
        ha.append(yh[-1][:, 0] - yl[-1][:, 0])
        acx.append((xl[-1] + xh[-1]) * 0.5)
    return xl, xh, yl, yh, acx, wa, ha


def _prep_inputs(pred0, pred1, pred2, anchors0, anchors1, anchors2,
                 boxes, labels):
    B = pred0.shape[0]
    xl, xh, yl, yh, acx, wa, ha = _host_geometry(anchors0, anchors1,
                                                 anchors2)
    area9 = np.array([wa[s] * ha[s] for s in range(3)], np.float32)  # [3,3]

    # [3, 224] concat over scales then tile -> [120, 224]
    def cat3(v):
        return np.concatenate([v[0], v[1], v[2]], axis=1)  # [3, 224]

    # rows ordered a-major: row = a * NBOX + j
    xl3 = np.repeat(cat3(xl), NBOX, axis=0).astype(np.float32)
    xh3 = np.repeat(cat3(xh), NBOX, axis=0).astype(np.float32)
    yl3 = np.repeat(cat3(yl), NBOX, axis=0).astype(np.float32)
    yh3 = np.repeat(cat3(yh), NBOX, axis=0).astype(np.float32)
    msk3 = np.repeat(np.eye(3, dtype=np.float16), NBOX, axis=0)  # [120, 3]

    boxes = np.asarray(boxes, np.float32)
    labels = np.asarray(labels)
    bx1, by1, bx2, by2 = (boxes[..., 0], boxes[..., 1], boxes[..., 2],
                          boxes[..., 3])
    bw = bx2 - bx1
    bh = by2 - by1
    barea = bw * bh + 1e-9
    bcx = bx1 + 0.5 * bw
    bcy = by1 + 0.5 * bh
    lnwb = np.log(bw)
    lnhb = np.log(bh)

    boxc = np.zeros((B, 120, 4), np.float32)
    sc3 = np.zeros((B, 120, 3), np.float32)
    bbrow = np.zeros((B, 1, 200), np.float32)
    boxt = np.zeros((B, NBOX, 8), np.float32)
    for bi in range(B):
        for a in range(3):
            pr = a * NBOX + np.arange(NBOX)
            boxc[bi, pr, 0] = bx1[bi]
            boxc[bi, pr, 1] = bx2[bi]
            boxc[bi, pr, 2] = by1[bi]
            boxc[bi, pr, 3] = by2[bi]
            for s in range(3):
                sc3[bi, pr, s] = QSC / (area9[s, a] + barea[bi])
        bbrow[bi, 0, 0:40] = bx1[bi]
        bbrow[bi, 0, 40:80] = bx2[bi]
        bbrow[bi, 0, 80:120] = by1[bi]
        bbrow[bi, 0, 120:160] = by2[bi]
        bbrow[bi, 0, 160:200] = barea[bi]
        boxt[bi, :, 0] = bcx[bi]
        boxt[bi, :, 1] = bcy[bi]
        boxt[bi, :, 2] = lnwb[bi]
        boxt[bi, :, 3] = lnhb[bi]
        boxt[bi, :, 4] = labels[bi].astype(np.float32)

    # PREDI [B, 128*672, NF]: row p*672+col
    # fields: 0-3 deltas, 4-6 cls, 7 xl, 8 xh, 9 yl, 10 yh, 11 acx,
    #         12 rwa, 13 rha, 14 lnwa, 15 lnha, 16 A
    predi = np.zeros((B, P, NANCH, NF), np.float32)
    preds = [np.asarray(pred0, np.float32), np.asarray(pred1, np.float32),
             np.asarray(pred2, np.float32)]
    for s, (W, Hs, co, cw, yo, ywd) in enumerate(SC):
        pr = preds[s].reshape(B, 3, 8, Hs, W)
        blk = np.transpose(pr, (0, 3, 1, 4, 2))  # [B, y, a, x, ch]
        # deltas 0-3 -> fields 0-3; cls 5-7 -> fields 4-6
        predi[:, 0:Hs, co:co + cw, 0:4] = \
            blk[..., 0:4].reshape(B, Hs, 3 * W, 4)
        predi[:, 0:Hs, co:co + cw, 4:7] = \
            blk[..., 5:8].reshape(B, Hs, 3 * W, 3)
        for a in range(3):
            c0, c1 = co + a * W, co + (a + 1) * W
            predi[:, :, c0:c1, 7] = xl[s][a][None, None, :]
            predi[:, :, c0:c1, 8] = xh[s][a][None, None, :]
            predi[:, 0:Hs, c0:c1, 9] = yl[s][a][None, :, None]
            predi[:, 0:Hs, c0:c1, 10] = yh[s][a][None, :, None]
            predi[:, :, c0:c1, 11] = acx[s][a][None, None, :]
            predi[:, :, c0:c1, 12] = 1.0 / wa[s][a]
            predi[:, :, c0:c1, 13] = 1.0 / ha[s][a]
            predi[:, :, c0:c1, 14] = np.log(wa[s][a])
            predi[:, :, c0:c1, 15] = np.log(ha[s][a])
            predi[:, :, c0:c1, 16] = area9[s, a]
    predi = predi.reshape(B, P * NANCH, NF)

    return dict(xl3=xl3, xh3=xh3, yl3=yl3, yh3=yh3, msk3=msk3,
                boxc=boxc, sc3=sc3, bbrow=bbrow, boxt=boxt, predi=predi)


def kernel(pred0, pred1, pred2, anchors0, anchors1, anchors2, boxes,
           labels, _want_results=False, _trace=False):
    nc = _get_compiled()
    hp = _prep_inputs(pred0, pred1, pred2, anchors0, anchors1, anchors2,
                      boxes, labels)
    in_maps = []
    for c in range(NCORES):
        sl = slice(c * SPC, (c + 1) * SPC)
        in_maps.append({
            "pred0": np.ascontiguousarray(pred0[sl], np.float32),
            "pred1": np.ascontiguousarray(pred1[sl], np.float32),
            "pred2": np.ascontiguousarray(pred2[sl], np.float32),
            "predi0": np.ascontiguousarray(hp["predi"][c * SPC]),
            "predi1": np.ascontiguousarray(hp["predi"][c * SPC + 1]),
            "boxc": np.ascontiguousarray(hp["boxc"][sl]),
            "sc3": np.ascontiguousarray(hp["sc3"][sl]),
            "bbrow": np.ascontiguousarray(hp["bbrow"][sl]),
            "boxt0": np.ascontiguousarray(hp["boxt"][c * SPC]),
            "boxt1": np.ascontiguousarray(hp["boxt"][c * SPC + 1]),
            "xl3": hp["xl3"], "xh3": hp["xh3"],
            "yl3": hp["yl3"], "yh3": hp["yh3"], "msk3": hp["msk3"],
        })
    res = bass_utils.run_bass_kernel_spmd(
        nc, in_maps, core_ids=list(range(NCORES)), trace=_trace)
    parts = np.stack([res.results[c]["out"][:, 0] for c in range(NCORES)])
    tot = parts.sum(axis=0, dtype=np.float64)
    tot_obj = tot_cls = tot_loc = tot_pos = tot_neg = 0.0
    for b in range(SPC):
        pb = b * 8
        tot_obj += tot[pb + 0] + tot[pb + 6] + tot[pb + 7]
        tot_cls += tot[pb + 1]
        tot_loc += tot[pb + 2]
        tot_pos += tot[pb + 3] + tot[pb + 4] + tot[pb + 5]
        tot_neg += tot[16 + b]
    norm = np.float32(max(tot_pos, 1.0))
    lo = np.float32(tot_obj / norm)
    lc = np.float32(tot_cls / norm)
    ll = np.float32(tot_loc / norm)
    ltot = np.float32(lo + lc + np.float32(2.0) * ll)
    out = (lo, lc, ll, ltot, np.float32(tot_pos), np.float32(tot_neg))
    out = tuple(np.asarray(v, np.float32) for v in out)
    if _want_results:
        return out, res
    return out
